# revision 12
# baseline (speedup 1.0000x reference)
# GCN encoder (DGI) forward on 8 Trainium2 NeuronCores.
#
# Node-partitioned (graph-parallel) sharding, bf16 message table:
#   - nodes are split contiguously across the 8 cores (N/8 per core)
#   - each core owns the edges whose *target* lands in its node range
#   - phase 1: every core computes xw' = dinv[s] * (x_s @ W_sn) in bf16 for
#     its own nodes (x is staged pre-transposed in bf16 so the matmul needs
#     no PE transposes), then an AllGather replicates the bf16 xw' table
#   - phase 2: each core gathers source rows for its edges with bulk
#     indirect DMA (256B bf16 rows), scatter-adds them into per-window PSUM
#     accumulators with one-hot selector matmuls on the PE (selectors built
#     in bf16 on DVE), folds the self-loop in as an identity-selector matmul
#     on the SBUF-resident phase-1 tiles, and runs the whole epilogue
#     (dinv[t] scale + PReLU) as a single ACT op per window.
#
# Host-side work is limited to index preprocessing (edge routing/sorting,
# degree counting, layout shuffles) and the tiny spectral-norm power
# iteration on W.

import numpy as np

import concourse.bacc as bacc
import concourse.bass as bass
import concourse.mybir as mybir
import concourse.tile as tile
from concourse.bass_utils import run_bass_kernel_spmd
from concourse.masks import make_identity

try:
    import ml_dtypes

    BF16 = np.dtype(ml_dtypes.bfloat16)
except ImportError:  # pragma: no cover
    BF16 = None

P = 128
F32 = mybir.dt.float32
BF16_T = mybir.dt.bfloat16
I16 = mybir.dt.int16

# test-harness hooks (ignored in grading): set TRACE=True before calling
# kernel() to capture an NTFF profile; the BassKernelResults lands in
# LAST_RESULT.
TRACE = False
LAST_RESULT = None


def _l2n(v, eps=1e-12):
    return v / (np.linalg.norm(v) + eps)


def _spectral_norm_host(W, u):
    W = W.astype(np.float32)
    u = u.astype(np.float32)
    v = _l2n(W.T @ u)
    u2 = _l2n(W @ v)
    sigma = np.float32(u2 @ (W @ v))
    return W / sigma


def _prep_host(n, edge_index, n_cores, win_group, nbuck, max_call_chunks):
    """Route edges to cores by target and build the SPMD chunk schedule.

    Chunks are 128 edges, each mapping into one 128-target window and one
    source bucket (dma_gather has int16 indices, so the gathered table is
    addressed in buckets of `bucket_rows` rows).  The table in DRAM is laid
    out bucket-major: bucket j holds, for every core c, the rows of c's
    nodes whose local id is in [j*spb, (j+1)*spb) — so bucket j is exactly
    the output of the j-th chunked AllGather and gathers on bucket j can
    start as soon as AG_j lands.  Chunk order: for each super-group of
    `win_group` windows, for each bucket, the chunks of the group's
    windows.  One dma_gather call covers one (group, bucket) run.
    Self-loops are NOT in the edge stream (folded in as identity-selector
    matmuls on the device).
    """
    assert n % n_cores == 0
    npc = n // n_cores
    nwin = -(-npc // P)
    assert npc % nbuck == 0
    spb = npc // nbuck  # rows each core contributes to one bucket
    bucket_rows = spb * n_cores
    assert bucket_rows < 32768

    row = np.ascontiguousarray(edge_index[0]).astype(np.int64)
    col = np.ascontiguousarray(edge_index[1]).astype(np.int64)

    # bucket-major table layout: node s lives in bucket (s%npc)//spb at row
    # (s//npc)*spb + s%spb
    sbuck = (row % npc) // spb
    srow = (row // npc) * spb + (row % spb)

    # sort all edges by (target window, source bucket) so each (core, window,
    # bucket) run is contiguous; target order within a chunk is free (tloc).
    wkey = (col // npc) * nwin + (col % npc) // P  # global window id
    key = wkey * nbuck + sbuck
    order = np.argsort(key, kind="stable")
    rs = srow[order]
    cs = col[order]
    cwb_sorted = key[order]

    deg = 1.0 + np.bincount(col, minlength=n).astype(np.float64)  # + self loop
    dinv_all = (deg ** -0.5).astype(np.float32)

    # counts per (core, window, bucket)
    cnt = np.bincount(key, minlength=n_cores * nwin * nbuck).reshape(
        n_cores, nwin, nbuck
    )
    kwb = -(-cnt // P)  # chunks per (c, w, b)
    kwb = kwb.max(axis=0)  # [nwin, nbuck] shared schedule

    # chunk order + gather-call runs
    chunk_win = []
    chunk_bucket = []
    call_sizes = []  # chunks per dma_gather call
    for wg in range(0, nwin, win_group):
        ws = range(wg, min(wg + win_group, nwin))
        for b in range(nbuck):
            r = int(sum(kwb[w, b] for w in ws))
            if r == 0:
                continue
            if max_call_chunks > 0:
                q = r
                while q > 0:
                    call_sizes.append(min(q, max_call_chunks))
                    q -= max_call_chunks
            else:
                call_sizes.append(r)
            for w in ws:
                chunk_win.extend([w] * kwb[w, b])
                chunk_bucket.extend([b] * kwb[w, b])
    chunk_win = np.asarray(chunk_win)
    chunk_bucket = np.asarray(chunk_bucket)
    nchunks = len(chunk_win)

    # first/last chunk per window in this order
    first_of_win = np.zeros(nchunks, bool)
    last_of_win = np.zeros(nchunks, bool)
    seen = set()
    for j in range(nchunks):
        w = int(chunk_win[j])
        if w not in seen:
            first_of_win[j] = True
            seen.add(w)
    seen = set()
    for j in range(nchunks - 1, -1, -1):
        w = int(chunk_win[j])
        if w not in seen:
            last_of_win[j] = True
            seen.add(w)

    # first destination chunk per (w, b)
    base_by_wb = {}
    for j in range(nchunks):
        key2 = (int(chunk_win[j]), int(chunk_bucket[j]))
        if key2 not in base_by_wb:
            base_by_wb[key2] = j

    # segment boundaries of (core, window, bucket) runs in the sorted list
    seg_lo_idx = np.searchsorted(
        cwb_sorted, np.arange(n_cores * nwin * nbuck), side="left"
    )
    seg_hi_idx = np.searchsorted(
        cwb_sorted, np.arange(n_cores * nwin * nbuck), side="right"
    )

    src_cores = []
    tloc_cores = []
    dinv_cores = []
    for c in range(n_cores):
        src_flat = np.zeros(nchunks * P, np.int16)
        tloc_flat = np.full(nchunks * P, -1.0, np.float32)
        for w in range(nwin):
            for b in range(nbuck):
                if (w, b) not in base_by_wb:
                    continue
                s = c * nwin * nbuck + w * nbuck + b
                i0, i1 = seg_lo_idx[s], seg_hi_idx[s]
                m = i1 - i0
                if m == 0:
                    continue
                d0 = base_by_wb[(w, b)] * P
                src_flat[d0 : d0 + m] = rs[i0:i1].astype(np.int16)
                tloc_flat[d0 : d0 + m] = (cs[i0:i1] - c * npc - w * P).astype(
                    np.float32
                )
        # dma_gather idx layout: idx i -> partition i%16, col i//16,
        # replicated over the 8 groups of 16 partitions.
        a = src_flat.reshape(nchunks, 8, 16)  # [j, p//16, p%16]
        a = np.transpose(a, (2, 0, 1)).reshape(16, nchunks * 8)
        src_cores.append(np.ascontiguousarray(np.tile(a, (8, 1))))
        tloc_cores.append(
            np.ascontiguousarray(tloc_flat.reshape(nchunks, P).T.astype(BF16))
        )

        dv = np.zeros(nwin * P, np.float32)
        dv[:npc] = dinv_all[c * npc : (c + 1) * npc]
        dinv_cores.append(np.ascontiguousarray(dv.reshape(nwin, P).T))

    return dict(
        npc=npc,
        nwin=nwin,
        nbuck=nbuck,
        spb=spb,
        bucket_rows=bucket_rows,
        nchunks=nchunks,
        chunk_win=chunk_win,
        chunk_bucket=chunk_bucket,
        call_sizes=call_sizes,
        first_of_win=first_of_win,
        last_of_win=last_of_win,
        src_cores=src_cores,
        tloc_cores=tloc_cores,
        dinv_cores=dinv_cores,
    )


def _build_nc(
    n,
    nfeat,
    nhid,
    n_cores,
    nwin,
    nbuck,
    bucket_rows,
    spb,
    nchunks,
    chunk_win,
    chunk_bucket,
    call_sizes,
    first_of_win,
    last_of_win,
    alpha,
    has_bias,
    gather_bufs=6,
    slab_wins=8,
    dma_scratch=32768,
):
    npc_pad = nwin * P
    npc = n // n_cores
    assert nfeat % P == 0
    nk = nfeat // P  # contraction tiles for x @ W

    nc = bacc.Bacc(
        "TRN2",
        target_bir_lowering=False,
        debug=False,
        enable_asserts=False,
        num_devices=n_cores,
        num_swdge_queues=4,
        dynamic_dma_scratch_size=dma_scratch,
    )

    # x staged pre-transposed+interleaved on host: x_in[p, k, m] = x[m, k*P+p]
    x_in = nc.dram_tensor("x_sh", [P, nk, npc_pad], BF16_T, kind="ExternalInput")
    w_in = nc.dram_tensor("w_sn", [nfeat, nhid], BF16_T, kind="ExternalInput")
    dinv_in = nc.dram_tensor("dinv", [P, nwin], F32, kind="ExternalInput")
    bias_in = nc.dram_tensor("bias_t", [P, nhid], F32, kind="ExternalInput")
    max_call = max(call_sizes)
    iota_in = nc.dram_tensor("iota_t", [P, max_call * P], BF16_T, kind="ExternalInput")
    src_in = nc.dram_tensor("src_idx", [P, nchunks * 8], I16, kind="ExternalInput")
    tloc_in = nc.dram_tensor("tloc", [P, nchunks], BF16_T, kind="ExternalInput")
    out_d = nc.dram_tensor("out_sh", [npc_pad, nhid], F32, kind="ExternalOutput")

    assert sum(call_sizes) == nchunks

    with tile.TileContext(nc) as tc:
        with (
            tc.tile_pool(name="consts", bufs=1) as cpool,
            tc.tile_pool(name="dram", bufs=1, space="DRAM") as dpool,
        ):
            # constants
            w_sb = cpool.tile([P, nk, nhid], BF16_T)
            nc.sync.dma_start(w_sb[:], w_in[:].rearrange("(k p) h -> p k h", p=P))
            bias_sb = cpool.tile([P, nhid], F32)
            nc.sync.dma_start(bias_sb[:], bias_in[:])
            iota_sb = cpool.tile([P, max_call * P], BF16_T)
            nc.sync.dma_start(iota_sb[:], iota_in[:])
            dinv_sb = cpool.tile([P, nwin], F32)
            nc.sync.dma_start(dinv_sb[:], dinv_in[:])
            ident = cpool.tile([P, P], BF16_T)
            make_identity(nc, ident[:])
            src_sb = cpool.tile([P, nchunks * 8], I16)
            nc.sync.dma_start(src_sb[:], src_in[:])
            tloc_sb = cpool.tile([P, nchunks], BF16_T)
            nc.sync.dma_start(tloc_sb[:], tloc_in[:])

            # phase-1 output kept resident in SBUF for the self-loop matmuls
            xwp_sb = cpool.tile([P, nwin * nhid], BF16_T)

            ag_in = dpool.tile([npc, nhid], BF16_T)
            ag_out = [
                dpool.tile(
                    [bucket_rows, nhid],
                    BF16_T,
                    addr_space="Shared",
                    name=f"ag_out{j}",
                )
                for j in range(nbuck)
            ]

            # ---- phase 1: xw' = dinv[s] * (x_s @ W_sn) for owned nodes ----
            # The AllGather is split into nbuck chunks; AG_j fires as soon as
            # the windows covering ag_in rows [j*spb, (j+1)*spb) are stored,
            # overlapping the collective with the rest of phase 1 and letting
            # bucket-j gathers start before the later chunks land.
            ag_trigger = {(-(-spb * (j + 1) // P)) - 1: j for j in range(nbuck)}
            with (
                tc.tile_pool(name="p1x", bufs=3) as xpool,
                tc.tile_pool(name="p1pm", bufs=4, space="PSUM") as psumXW,
            ):
                for s0 in range(0, nwin, slab_wins):
                    ns = min(slab_wins, nwin - s0)
                    xt = xpool.tile([P, nk, slab_wins * P], BF16_T)
                    nc.sync.dma_start(
                        xt[:, :, : ns * P],
                        x_in[:, :, s0 * P : (s0 + ns) * P],
                    )
                    for wr in range(ns):
                        w = s0 + wr
                        nrow = min(P, npc - w * P)
                        pxw = psumXW.tile([P, nhid], F32)
                        for k in range(nk):
                            nc.tensor.matmul(
                                pxw[:],
                                lhsT=xt[:, k, wr * P : (wr + 1) * P],
                                rhs=w_sb[:, k, :],
                                start=(k == 0),
                                stop=(k == nk - 1),
                            )
                        seg = xwp_sb[:, w * nhid : (w + 1) * nhid]
                        nc.scalar.activation(
                            out=seg,
                            in_=pxw[:],
                            func=mybir.ActivationFunctionType.Copy,
                            scale=dinv_sb[:, w : w + 1],
                        )
                        nc.sync.dma_start(
                            ag_in[w * P : w * P + nrow, :], seg[:nrow]
                        )
                        if w in ag_trigger:
                            j = ag_trigger[w]
                            nc.gpsimd.collective_compute(
                                "AllGather",
                                mybir.AluOpType.bypass,
                                replica_groups=[list(range(n_cores))],
                                ins=[ag_in[j * spb : (j + 1) * spb]],
                                outs=[ag_out[j][:]],
                            )

            # ---- phase 2: gather + one-hot matmul scatter-add + epilogue ----
            psum_by_win = {}
            with (
                tc.tile_pool(name="gat", bufs=gather_bufs) as gpool,
                tc.tile_pool(name="sel", bufs=6) as spool,
                tc.tile_pool(name="og", bufs=4) as opool,
                tc.tile_pool(name="acc", bufs=8, space="PSUM") as ppool,
            ):
                j = 0
                for ci, r in enumerate(call_sizes):
                    gbuf = gpool.tile(
                        [P, max_call * nhid], BF16_T, tag="gbuf", name="gbuf"
                    )
                    b = int(chunk_bucket[j])
                    nc.gpsimd.dma_gather(
                        gbuf[:, : r * nhid].rearrange("p (k e) -> p k e", e=nhid),
                        ag_out[b][:],
                        src_sb[:, j * 8 : (j + r) * 8],
                        r * P,
                        r * P,
                        nhid,
                        queue_num=ci % 4,
                    )
                    # one-hot selectors for the whole call in one DVE op
                    sel_big = spool.tile(
                        [P, max_call * P], BF16_T, tag="sel", name="sel_big"
                    )
                    nc.vector.tensor_tensor(
                        out=sel_big[:, : r * P].rearrange("p (k e) -> p k e", e=P),
                        in0=tloc_sb[:, j : j + r].to_broadcast([P, r, P]),
                        in1=iota_sb[:, : r * P].rearrange("p (k e) -> p k e", e=P),
                        op=mybir.AluOpType.is_equal,
                    )
                    for kk in range(r):
                        w = int(chunk_win[j])
                        if first_of_win[j]:
                            pw = ppool.tile([P, nhid], F32, tag="pw", name="pw")
                            psum_by_win[w] = pw
                            # self-loop: identity selector over the resident
                            # phase-1 tile (start=True resets the bank)
                            nc.tensor.matmul(
                                pw[:],
                                lhsT=ident[:],
                                rhs=xwp_sb[:, w * nhid : (w + 1) * nhid],
                                start=True,
                                stop=False,
                            )
                        pw = psum_by_win[w]
                        nc.tensor.matmul(
                            pw[:],
                            lhsT=sel_big[:, kk * P : (kk + 1) * P],
                            rhs=gbuf[:, kk * nhid : (kk + 1) * nhid],
                            start=False,
                            stop=bool(last_of_win[j]),
                        )
                        if last_of_win[j]:
                            og = opool.tile([P, nhid], F32, tag="og", name="og")
                            nrow = min(P, npc - w * P)
                            if has_bias:
                                nc.scalar.activation(
                                    out=og[:],
                                    in_=pw[:],
                                    func=mybir.ActivationFunctionType.Copy,
                                    scale=dinv_sb[:, w : w + 1],
                                )
                                nc.vector.tensor_tensor(
                                    out=og[:],
                                    in0=og[:],
                                    in1=bias_sb[:],
                                    op=mybir.AluOpType.add,
                                )
                                t2 = opool.tile(
                                    [P, nhid], F32, tag="t2", name="t2"
                                )
                                nc.vector.tensor_scalar(
                                    out=t2[:],
                                    in0=og[:],
                                    scalar1=0.0,
                                    scalar2=float(alpha),
                                    op0=mybir.AluOpType.min,
                                    op1=mybir.AluOpType.mult,
                                )
                                nc.vector.tensor_scalar_max(og[:], og[:], 0.0)
                                nc.vector.tensor_tensor(
                                    out=og[:],
                                    in0=og[:],
                                    in1=t2[:],
                                    op=mybir.AluOpType.add,
                                )
                            else:
                                # out = PReLU(dinv[t] * agg), one ACT op
                                nc.scalar.activation(
                                    out=og[:],
                                    in_=pw[:],
                                    func=mybir.ActivationFunctionType.Prelu,
                                    scale=dinv_sb[:, w : w + 1],
                                    alpha=float(alpha),
                                )
                            nc.sync.dma_start(
                                out_d[w * P : w * P + nrow, :], og[:nrow]
                            )
                        j += 1

    nc.compile()
    return nc


def kernel(**inputs):
    x = np.asarray(inputs["x"], dtype=np.float32)
    edge_index = np.asarray(inputs["edge_index"])
    W = np.asarray(inputs["W"], dtype=np.float32)
    bias = np.asarray(inputs["bias"], dtype=np.float32)
    prelu_a = np.asarray(inputs["prelu_a"], dtype=np.float32)
    u = np.asarray(inputs["u"], dtype=np.float32)

    n, nfeat = x.shape
    nhid = W.shape[1]
    n_cores = 8
    win_group = 4
    nbuck = -(-n // 32767)  # int16 index reach per dma_gather bucket
    alpha = float(prelu_a.reshape(-1)[0])
    has_bias = bool(np.any(bias != 0.0))

    # one dma_gather call must stay under the SWDGE ring carveout
    # (dynamic_dma_scratch_size//16 descriptors); 7 chunks = 896 < 1024
    max_call_chunks = 7
    dma_scratch = 16384

    npc = n // n_cores
    nwin = -(-npc // P)
    npc_pad = nwin * P
    nk = nfeat // P

    w_sn = _spectral_norm_host(W, u)
    prep = _prep_host(n, edge_index, n_cores, win_group, nbuck, max_call_chunks)
    nchunks = prep["nchunks"]

    nc = _build_nc(
        n,
        nfeat,
        nhid,
        n_cores,
        nwin,
        prep["nbuck"],
        prep["bucket_rows"],
        prep["spb"],
        nchunks,
        prep["chunk_win"],
        prep["chunk_bucket"],
        prep["call_sizes"],
        prep["first_of_win"],
        prep["last_of_win"],
        alpha,
        has_bias,
        dma_scratch=dma_scratch,
    )

    bias_t = np.ascontiguousarray(np.tile(bias[None, :], (P, 1)))
    max_call = max(prep["call_sizes"])
    iota_t = np.ascontiguousarray(
        np.tile(
            np.tile(np.arange(P, dtype=np.float32), max_call)[None, :], (P, 1)
        ).astype(BF16)
    )
    w_bf = np.ascontiguousarray(w_sn.astype(BF16))

    in_maps = []
    for c in range(n_cores):
        xp = np.zeros((npc_pad, nfeat), np.float32)
        xp[:npc] = x[c * npc : (c + 1) * npc]
        # x_sh[p, k, m] = xp[m, k*P + p]
        x_sh = np.ascontiguousarray(
            xp.T.reshape(nk, P, npc_pad).transpose(1, 0, 2).astype(BF16)
        )
        in_maps.append(
            {
                "x_sh": x_sh,
                "w_sn": w_bf,
                "dinv": prep["dinv_cores"][c],
                "bias_t": bias_t,
                "iota_t": iota_t,
                "src_idx": prep["src_cores"][c],
                "tloc": prep["tloc_cores"][c],
            }
        )

    res = run_bass_kernel_spmd(
        nc, in_maps, core_ids=list(range(n_cores)), trace=TRACE
    )
    global LAST_RESULT
    LAST_RESULT = res
    out = np.concatenate(
        [res.results[c]["out_sh"][:npc] for c in range(n_cores)], axis=0
    )
    return out


# revision 20
# speedup vs baseline: 1.1840x; 1.1840x over previous
# GCN encoder (DGI) forward on 8 Trainium2 NeuronCores.
#
# Node-partitioned (graph-parallel) sharding, bf16 message table:
#   - nodes are split contiguously across the 8 cores (N/8 per core)
#   - each core owns the edges whose *target* lands in its node range
#   - phase 1: every core computes xw' = dinv[s] * (x_s @ W_sn) in bf16 for
#     its own nodes (x is staged pre-transposed in bf16 so the matmul needs
#     no PE transposes), then an AllGather replicates the bf16 xw' table
#   - phase 2: each core gathers source rows for its edges with bulk
#     indirect DMA (256B bf16 rows), scatter-adds them into per-window PSUM
#     accumulators with one-hot selector matmuls on the PE (selectors built
#     in bf16 on DVE), folds the self-loop in as an identity-selector matmul
#     on the SBUF-resident phase-1 tiles, and runs the whole epilogue
#     (dinv[t] scale + PReLU) as a single ACT op per window.
#
# Host-side work is limited to index preprocessing (edge routing/sorting,
# degree counting, layout shuffles) and the tiny spectral-norm power
# iteration on W.

import numpy as np

import concourse.bacc as bacc
import concourse.bass as bass
import concourse.mybir as mybir
import concourse.tile as tile
from concourse.bass_utils import run_bass_kernel_spmd
from concourse.masks import make_identity

try:
    import ml_dtypes

    BF16 = np.dtype(ml_dtypes.bfloat16)
except ImportError:  # pragma: no cover
    BF16 = None

P = 128
F32 = mybir.dt.float32
BF16_T = mybir.dt.bfloat16
I16 = mybir.dt.int16

# test-harness hooks (ignored in grading): set TRACE=True before calling
# kernel() to capture an NTFF profile; the BassKernelResults lands in
# LAST_RESULT.
TRACE = False
LAST_RESULT = None


def _l2n(v, eps=1e-12):
    return v / (np.linalg.norm(v) + eps)


def _spectral_norm_host(W, u):
    W = W.astype(np.float32)
    u = u.astype(np.float32)
    v = _l2n(W.T @ u)
    u2 = _l2n(W @ v)
    sigma = np.float32(u2 @ (W @ v))
    return W / sigma


def _prep_host(n, edge_index, n_cores, win_group, nbuck, max_call_chunks):
    """Route edges to cores by target and build the SPMD chunk schedule.

    Chunks are 128 edges, each mapping into one 128-target window and one
    source bucket (dma_gather has int16 indices, so the gathered table is
    addressed in buckets of `bucket_rows` rows).  The table in DRAM is laid
    out bucket-major: bucket j holds, for every core c, the rows of c's
    nodes whose local id is in [j*spb, (j+1)*spb) — so bucket j is exactly
    the output of the j-th chunked AllGather and gathers on bucket j can
    start as soon as AG_j lands.  Chunk order: for each super-group of
    `win_group` windows, for each bucket, the chunks of the group's
    windows.  One dma_gather call covers one (group, bucket) run.
    Self-loops are NOT in the edge stream (folded in as identity-selector
    matmuls on the device).
    """
    assert n % n_cores == 0
    npc = n // n_cores
    nwin = -(-npc // P)
    assert npc % nbuck == 0
    spb = npc // nbuck
    bucket_rows = -(-n // nbuck)
    assert bucket_rows < 32768

    row = np.ascontiguousarray(edge_index[0]).astype(np.int64)
    col = np.ascontiguousarray(edge_index[1]).astype(np.int64)

    sbuck = row // bucket_rows
    srow = row % bucket_rows

    # sort all edges by (target window, source bucket) so each (core, window,
    # bucket) run is contiguous; target order within a chunk is free (tloc).
    wkey = (col // npc) * nwin + (col % npc) // P  # global window id
    key = wkey * nbuck + sbuck
    order = np.argsort(key, kind="stable")
    rs = srow[order]
    cs = col[order]
    cwb_sorted = key[order]

    deg = 1.0 + np.bincount(col, minlength=n).astype(np.float64)  # + self loop
    dinv_all = (deg ** -0.5).astype(np.float32)

    # counts per (core, window, bucket)
    cnt = np.bincount(key, minlength=n_cores * nwin * nbuck).reshape(
        n_cores, nwin, nbuck
    )
    kwb = -(-cnt // P)  # chunks per (c, w, b)
    kwb = kwb.max(axis=0)  # [nwin, nbuck] shared schedule

    # chunk order + gather-call runs.  Each call covers chunks of ONE
    # (window, bucket) cell so that every call's padded tail is a run of
    # negative indices, which the DGE skips (no descriptors, no bytes).
    chunk_win = []
    chunk_bucket = []
    call_sizes = []  # chunks per dma_gather call
    for wg in range(0, nwin, win_group):
        ws = range(wg, min(wg + win_group, nwin))
        for b in range(nbuck):
            for w in ws:
                r = int(kwb[w, b])
                if r == 0:
                    continue
                q = r
                while q > 0:
                    call_sizes.append(min(q, max_call_chunks))
                    q -= max_call_chunks
                chunk_win.extend([w] * r)
                chunk_bucket.extend([b] * r)
    chunk_win = np.asarray(chunk_win)
    chunk_bucket = np.asarray(chunk_bucket)
    nchunks = len(chunk_win)

    # first/last chunk per window in this order
    first_of_win = np.zeros(nchunks, bool)
    last_of_win = np.zeros(nchunks, bool)
    seen = set()
    for j in range(nchunks):
        w = int(chunk_win[j])
        if w not in seen:
            first_of_win[j] = True
            seen.add(w)
    seen = set()
    for j in range(nchunks - 1, -1, -1):
        w = int(chunk_win[j])
        if w not in seen:
            last_of_win[j] = True
            seen.add(w)

    # first destination chunk per (w, b)
    base_by_wb = {}
    for j in range(nchunks):
        key2 = (int(chunk_win[j]), int(chunk_bucket[j]))
        if key2 not in base_by_wb:
            base_by_wb[key2] = j

    # segment boundaries of (core, window, bucket) runs in the sorted list
    seg_lo_idx = np.searchsorted(
        cwb_sorted, np.arange(n_cores * nwin * nbuck), side="left"
    )
    seg_hi_idx = np.searchsorted(
        cwb_sorted, np.arange(n_cores * nwin * nbuck), side="right"
    )

    src_cores = []
    tloc_cores = []
    dinv_cores = []
    for c in range(n_cores):
        src_flat = np.zeros(nchunks * P, np.int16)
        tloc_flat = np.full(nchunks * P, -1.0, np.float32)
        for w in range(nwin):
            for b in range(nbuck):
                if (w, b) not in base_by_wb:
                    continue
                s = c * nwin * nbuck + w * nbuck + b
                i0, i1 = seg_lo_idx[s], seg_hi_idx[s]
                m = i1 - i0
                if m == 0:
                    continue
                d0 = base_by_wb[(w, b)] * P
                src_flat[d0 : d0 + m] = rs[i0:i1].astype(np.int16)
                tloc_flat[d0 : d0 + m] = (cs[i0:i1] - c * npc - w * P).astype(
                    np.float32
                )
        # dma_gather idx layout: idx i -> partition i%16, col i//16,
        # replicated over the 8 groups of 16 partitions.
        a = src_flat.reshape(nchunks, 8, 16)  # [j, p//16, p%16]
        a = np.transpose(a, (2, 0, 1)).reshape(16, nchunks * 8)
        src_cores.append(np.ascontiguousarray(np.tile(a, (8, 1))))
        tloc_cores.append(
            np.ascontiguousarray(tloc_flat.reshape(nchunks, P).T.astype(BF16))
        )

        dv = np.zeros(nwin * P, np.float32)
        dv[:npc] = dinv_all[c * npc : (c + 1) * npc]
        dinv_cores.append(np.ascontiguousarray(dv.reshape(nwin, P).T))

    return dict(
        npc=npc,
        nwin=nwin,
        nbuck=nbuck,
        spb=spb,
        bucket_rows=bucket_rows,
        nchunks=nchunks,
        chunk_win=chunk_win,
        chunk_bucket=chunk_bucket,
        call_sizes=call_sizes,
        first_of_win=first_of_win,
        last_of_win=last_of_win,
        src_cores=src_cores,
        tloc_cores=tloc_cores,
        dinv_cores=dinv_cores,
    )


def _build_nc(
    n,
    nfeat,
    nhid,
    n_cores,
    nwin,
    nbuck,
    bucket_rows,
    spb,
    nchunks,
    chunk_win,
    chunk_bucket,
    call_sizes,
    first_of_win,
    last_of_win,
    alpha,
    has_bias,
    gather_bufs=6,
    slab_wins=8,
    dma_scratch=32768,
):
    npc_pad = nwin * P
    npc = n // n_cores
    assert nfeat % P == 0
    nk = nfeat // P  # contraction tiles for x @ W

    nc = bacc.Bacc(
        "TRN2",
        target_bir_lowering=False,
        debug=False,
        enable_asserts=False,
        num_devices=n_cores,
        num_swdge_queues=4,
        dynamic_dma_scratch_size=dma_scratch,
    )

    # x staged pre-transposed+interleaved on host: x_in[p, k, m] = x[m, k*P+p]
    x_in = nc.dram_tensor("x_sh", [P, nk, npc_pad], BF16_T, kind="ExternalInput")
    w_in = nc.dram_tensor("w_sn", [nfeat, nhid], BF16_T, kind="ExternalInput")
    dinv_in = nc.dram_tensor("dinv", [P, nwin], F32, kind="ExternalInput")
    bias_in = nc.dram_tensor("bias_t", [P, nhid], F32, kind="ExternalInput")
    max_call = max(call_sizes)
    iota_in = nc.dram_tensor("iota_t", [P, max_call * P], BF16_T, kind="ExternalInput")
    src_in = nc.dram_tensor("src_idx", [P, nchunks * 8], I16, kind="ExternalInput")
    tloc_in = nc.dram_tensor("tloc", [P, nchunks], BF16_T, kind="ExternalInput")
    out_d = nc.dram_tensor("out_sh", [npc_pad, nhid], F32, kind="ExternalOutput")

    assert sum(call_sizes) == nchunks

    with tile.TileContext(nc) as tc:
        with (
            tc.tile_pool(name="consts", bufs=1) as cpool,
            tc.tile_pool(name="dram", bufs=1, space="DRAM") as dpool,
        ):
            # constants
            w_sb = cpool.tile([P, nk, nhid], BF16_T)
            nc.sync.dma_start(w_sb[:], w_in[:].rearrange("(k p) h -> p k h", p=P))
            bias_sb = cpool.tile([P, nhid], F32)
            nc.sync.dma_start(bias_sb[:], bias_in[:])
            iota_sb = cpool.tile([P, max_call * P], BF16_T)
            nc.sync.dma_start(iota_sb[:], iota_in[:])
            dinv_sb = cpool.tile([P, nwin], F32)
            nc.sync.dma_start(dinv_sb[:], dinv_in[:])
            ident = cpool.tile([P, P], BF16_T)
            make_identity(nc, ident[:])
            src_sb = cpool.tile([P, nchunks * 8], I16)
            nc.sync.dma_start(src_sb[:], src_in[:])
            tloc_sb = cpool.tile([P, nchunks], BF16_T)
            nc.sync.dma_start(tloc_sb[:], tloc_in[:])

            # phase-1 output kept resident in SBUF for the self-loop matmuls
            xwp_sb = cpool.tile([P, nwin * nhid], BF16_T)

            ag_in = dpool.tile([npc, nhid], BF16_T)
            ag_out = dpool.tile([n, nhid], BF16_T, addr_space="Shared")

            # ---- phase 1: xw' = dinv[s] * (x_s @ W_sn) for owned nodes ----
            with (
                tc.tile_pool(name="p1x", bufs=3) as xpool,
                tc.tile_pool(name="p1pm", bufs=4, space="PSUM") as psumXW,
            ):
                for s0 in range(0, nwin, slab_wins):
                    ns = min(slab_wins, nwin - s0)
                    xt = xpool.tile([P, nk, slab_wins * P], BF16_T)
                    nc.sync.dma_start(
                        xt[:, :, : ns * P],
                        x_in[:, :, s0 * P : (s0 + ns) * P],
                    )
                    for wr in range(ns):
                        w = s0 + wr
                        nrow = min(P, npc - w * P)
                        pxw = psumXW.tile([P, nhid], F32)
                        for k in range(nk):
                            nc.tensor.matmul(
                                pxw[:],
                                lhsT=xt[:, k, wr * P : (wr + 1) * P],
                                rhs=w_sb[:, k, :],
                                start=(k == 0),
                                stop=(k == nk - 1),
                            )
                        seg = xwp_sb[:, w * nhid : (w + 1) * nhid]
                        nc.scalar.activation(
                            out=seg,
                            in_=pxw[:],
                            func=mybir.ActivationFunctionType.Copy,
                            scale=dinv_sb[:, w : w + 1],
                        )
                        nc.sync.dma_start(
                            ag_in[w * P : w * P + nrow, :], seg[:nrow]
                        )

            nc.gpsimd.collective_compute(
                "AllGather",
                mybir.AluOpType.bypass,
                replica_groups=[list(range(n_cores))],
                ins=[ag_in[:]],
                outs=[ag_out[:]],
            )

            # ---- phase 2: gather + one-hot matmul scatter-add + epilogue ----
            psum_by_win = {}
            with (
                tc.tile_pool(name="gat", bufs=gather_bufs) as gpool,
                tc.tile_pool(name="sel", bufs=6) as spool,
                tc.tile_pool(name="og", bufs=4) as opool,
                tc.tile_pool(name="acc", bufs=8, space="PSUM") as ppool,
            ):
                j = 0
                for ci, r in enumerate(call_sizes):
                    gbuf = gpool.tile(
                        [P, max_call * nhid], BF16_T, tag="gbuf", name="gbuf"
                    )
                    if ci < gather_bufs:
                        # skipped (negative-idx) tail slots are never written;
                        # zero once so stale SBUF can't inject NaN into matmuls
                        nc.vector.memset(gbuf[:], 0.0)
                    b = int(chunk_bucket[j])
                    rows = min(bucket_rows, n - b * bucket_rows)
                    nc.gpsimd.dma_gather(
                        gbuf[:, : r * nhid].rearrange("p (k e) -> p k e", e=nhid),
                        ag_out[b * bucket_rows : b * bucket_rows + rows, :],
                        src_sb[:, j * 8 : (j + r) * 8],
                        r * P,
                        r * P,
                        nhid,
                        queue_num=ci % 4,
                    )
                    # one-hot selectors for the whole call in one DVE op
                    sel_big = spool.tile(
                        [P, max_call * P], BF16_T, tag="sel", name="sel_big"
                    )
                    nc.vector.tensor_tensor(
                        out=sel_big[:, : r * P].rearrange("p (k e) -> p k e", e=P),
                        in0=tloc_sb[:, j : j + r].to_broadcast([P, r, P]),
                        in1=iota_sb[:, : r * P].rearrange("p (k e) -> p k e", e=P),
                        op=mybir.AluOpType.is_equal,
                    )
                    for kk in range(r):
                        w = int(chunk_win[j])
                        if first_of_win[j]:
                            pw = ppool.tile([P, nhid], F32, tag="pw", name="pw")
                            psum_by_win[w] = pw
                            # self-loop: identity selector over the resident
                            # phase-1 tile (start=True resets the bank)
                            nc.tensor.matmul(
                                pw[:],
                                lhsT=ident[:],
                                rhs=xwp_sb[:, w * nhid : (w + 1) * nhid],
                                start=True,
                                stop=False,
                            )
                        pw = psum_by_win[w]
                        nc.tensor.matmul(
                            pw[:],
                            lhsT=sel_big[:, kk * P : (kk + 1) * P],
                            rhs=gbuf[:, kk * nhid : (kk + 1) * nhid],
                            start=False,
                            stop=bool(last_of_win[j]),
                        )
                        if last_of_win[j]:
                            og = opool.tile([P, nhid], F32, tag="og", name="og")
                            nrow = min(P, npc - w * P)
                            if has_bias:
                                nc.scalar.activation(
                                    out=og[:],
                                    in_=pw[:],
                                    func=mybir.ActivationFunctionType.Copy,
                                    scale=dinv_sb[:, w : w + 1],
                                )
                                nc.vector.tensor_tensor(
                                    out=og[:],
                                    in0=og[:],
                                    in1=bias_sb[:],
                                    op=mybir.AluOpType.add,
                                )
                                t2 = opool.tile(
                                    [P, nhid], F32, tag="t2", name="t2"
                                )
                                nc.vector.tensor_scalar(
                                    out=t2[:],
                                    in0=og[:],
                                    scalar1=0.0,
                                    scalar2=float(alpha),
                                    op0=mybir.AluOpType.min,
                                    op1=mybir.AluOpType.mult,
                                )
                                nc.vector.tensor_scalar_max(og[:], og[:], 0.0)
                                nc.vector.tensor_tensor(
                                    out=og[:],
                                    in0=og[:],
                                    in1=t2[:],
                                    op=mybir.AluOpType.add,
                                )
                            else:
                                # out = PReLU(dinv[t] * agg), one ACT op
                                nc.scalar.activation(
                                    out=og[:],
                                    in_=pw[:],
                                    func=mybir.ActivationFunctionType.Prelu,
                                    scale=dinv_sb[:, w : w + 1],
                                    alpha=float(alpha),
                                )
                            nc.sync.dma_start(
                                out_d[w * P : w * P + nrow, :], og[:nrow]
                            )
                        j += 1

    nc.compile()
    return nc


def kernel(**inputs):
    x = np.asarray(inputs["x"], dtype=np.float32)
    edge_index = np.asarray(inputs["edge_index"])
    W = np.asarray(inputs["W"], dtype=np.float32)
    bias = np.asarray(inputs["bias"], dtype=np.float32)
    prelu_a = np.asarray(inputs["prelu_a"], dtype=np.float32)
    u = np.asarray(inputs["u"], dtype=np.float32)

    n, nfeat = x.shape
    nhid = W.shape[1]
    n_cores = 8
    win_group = 4
    nbuck = -(-n // 32767)  # int16 index reach per dma_gather bucket
    alpha = float(prelu_a.reshape(-1)[0])
    has_bias = bool(np.any(bias != 0.0))

    # one dma_gather call must stay under the SWDGE ring carveout
    # (dynamic_dma_scratch_size//16 descriptors); 7 chunks = 896 < 1024
    max_call_chunks = 7
    dma_scratch = 16384

    npc = n // n_cores
    nwin = -(-npc // P)
    npc_pad = nwin * P
    nk = nfeat // P

    w_sn = _spectral_norm_host(W, u)
    prep = _prep_host(n, edge_index, n_cores, win_group, nbuck, max_call_chunks)
    nchunks = prep["nchunks"]

    nc = _build_nc(
        n,
        nfeat,
        nhid,
        n_cores,
        nwin,
        prep["nbuck"],
        prep["bucket_rows"],
        prep["spb"],
        nchunks,
        prep["chunk_win"],
        prep["chunk_bucket"],
        prep["call_sizes"],
        prep["first_of_win"],
        prep["last_of_win"],
        alpha,
        has_bias,
        dma_scratch=dma_scratch,
    )

    bias_t = np.ascontiguousarray(np.tile(bias[None, :], (P, 1)))
    max_call = max(prep["call_sizes"])
    iota_t = np.ascontiguousarray(
        np.tile(
            np.tile(np.arange(P, dtype=np.float32), max_call)[None, :], (P, 1)
        ).astype(BF16)
    )
    w_bf = np.ascontiguousarray(w_sn.astype(BF16))

    in_maps = []
    for c in range(n_cores):
        xp = np.zeros((npc_pad, nfeat), np.float32)
        xp[:npc] = x[c * npc : (c + 1) * npc]
        # x_sh[p, k, m] = xp[m, k*P + p]
        x_sh = np.ascontiguousarray(
            xp.T.reshape(nk, P, npc_pad).transpose(1, 0, 2).astype(BF16)
        )
        in_maps.append(
            {
                "x_sh": x_sh,
                "w_sn": w_bf,
                "dinv": prep["dinv_cores"][c],
                "bias_t": bias_t,
                "iota_t": iota_t,
                "src_idx": prep["src_cores"][c],
                "tloc": prep["tloc_cores"][c],
            }
        )

    res = run_bass_kernel_spmd(
        nc, in_maps, core_ids=list(range(n_cores)), trace=TRACE
    )
    global LAST_RESULT
    LAST_RESULT = res
    out = np.concatenate(
        [res.results[c]["out_sh"][:npc] for c in range(n_cores)], axis=0
    )
    return out


# revision 28
# speedup vs baseline: 1.3531x; 1.1428x over previous
# GCN encoder (DGI) forward on 8 Trainium2 NeuronCores.
#
# Node-partitioned (graph-parallel) sharding, bf16 message table:
#   - nodes are split contiguously across the 8 cores (N/8 per core)
#   - each core owns the edges whose *target* lands in its node range
#   - phase 1: every core computes xw' = dinv[s] * (x_s @ W_sn) in bf16 for
#     its own nodes (x is staged pre-transposed in bf16 so the matmul needs
#     no PE transposes), then an AllGather replicates the bf16 xw' table
#   - phase 2: each core gathers source rows for its edges with bulk
#     indirect DMA (256B bf16 rows), scatter-adds them into per-window PSUM
#     accumulators with one-hot selector matmuls on the PE (selectors built
#     in bf16 on DVE), folds the self-loop in as an identity-selector matmul
#     on the SBUF-resident phase-1 tiles, and runs the whole epilogue
#     (dinv[t] scale + PReLU) as a single ACT op per window.
#
# Host-side work is limited to index preprocessing (edge routing/sorting,
# degree counting, layout shuffles) and the tiny spectral-norm power
# iteration on W.

import numpy as np

import concourse.bacc as bacc
import concourse.bass as bass
import concourse.mybir as mybir
import concourse.tile as tile
from concourse.bass_utils import run_bass_kernel_spmd
from concourse.masks import make_identity

try:
    import ml_dtypes

    BF16 = np.dtype(ml_dtypes.bfloat16)
except ImportError:  # pragma: no cover
    BF16 = None

P = 128
F32 = mybir.dt.float32
BF16_T = mybir.dt.bfloat16
I16 = mybir.dt.int16

# test-harness hooks (ignored in grading): set TRACE=True before calling
# kernel() to capture an NTFF profile; the BassKernelResults lands in
# LAST_RESULT.
TRACE = False
LAST_RESULT = None


def _l2n(v, eps=1e-12):
    return v / (np.linalg.norm(v) + eps)


def _spectral_norm_host(W, u):
    W = W.astype(np.float32)
    u = u.astype(np.float32)
    v = _l2n(W.T @ u)
    u2 = _l2n(W @ v)
    sigma = np.float32(u2 @ (W @ v))
    return W / sigma


def _prep_host(n, edge_index, n_cores, win_group, nbuck, max_call_chunks):
    """Route edges to cores by target and build the SPMD chunk schedule.

    Chunks are 128 edges, each mapping into one 128-target window and one
    source bucket (dma_gather has int16 indices, so the gathered table is
    addressed in buckets of `bucket_rows` rows).  The table in DRAM is laid
    out bucket-major: bucket j holds, for every core c, the rows of c's
    nodes whose local id is in [j*spb, (j+1)*spb) — so bucket j is exactly
    the output of the j-th chunked AllGather and gathers on bucket j can
    start as soon as AG_j lands.  Chunk order: for each super-group of
    `win_group` windows, for each bucket, the chunks of the group's
    windows.  One dma_gather call covers one (group, bucket) run.
    Self-loops are NOT in the edge stream (folded in as identity-selector
    matmuls on the device).
    """
    assert n % n_cores == 0
    npc = n // n_cores
    nwin = -(-npc // P)
    assert npc % nbuck == 0
    spb = npc // nbuck
    bucket_rows = -(-n // nbuck)
    assert bucket_rows < 32768

    row = np.ascontiguousarray(edge_index[0]).astype(np.int64)
    col = np.ascontiguousarray(edge_index[1]).astype(np.int64)

    sbuck = row // bucket_rows
    srow = row % bucket_rows

    # sort all edges by (target window, source bucket) so each (core, window,
    # bucket) run is contiguous; target order within a chunk is free (tloc).
    wkey = (col // npc) * nwin + (col % npc) // P  # global window id
    key = wkey * nbuck + sbuck
    order = np.argsort(key, kind="stable")
    rs = srow[order]
    cs = col[order]
    cwb_sorted = key[order]

    deg = 1.0 + np.bincount(col, minlength=n).astype(np.float64)  # + self loop
    dinv_all = (deg ** -0.5).astype(np.float32)

    # counts per (core, window, bucket)
    cnt = np.bincount(key, minlength=n_cores * nwin * nbuck).reshape(
        n_cores, nwin, nbuck
    )
    kwb = -(-cnt // P)  # chunks per (c, w, b)
    kwb = kwb.max(axis=0)  # [nwin, nbuck] shared schedule

    # chunk order + gather-call runs.  Each call covers chunks of ONE
    # (window, bucket) cell so that every call's padded tail is a run of
    # negative indices; with num_idxs_reg = the per-core true count the DGE
    # skips the tail entirely (no descriptors, no bytes).
    chunk_win = []
    chunk_bucket = []
    call_sizes = []  # chunks per dma_gather call
    call_meta = []  # (w, b, q0) chunk offset of the call within its cell
    for wg in range(0, nwin, win_group):
        ws = range(wg, min(wg + win_group, nwin))
        for b in range(nbuck):
            for w in ws:
                r = int(kwb[w, b])
                if r == 0:
                    continue
                q0 = 0
                while q0 < r:
                    call_sizes.append(min(r - q0, max_call_chunks))
                    call_meta.append((w, b, q0))
                    q0 += max_call_chunks
                chunk_win.extend([w] * r)
                chunk_bucket.extend([b] * r)
    chunk_win = np.asarray(chunk_win)
    chunk_bucket = np.asarray(chunk_bucket)
    nchunks = len(chunk_win)

    # first/last chunk per window in this order
    first_of_win = np.zeros(nchunks, bool)
    last_of_win = np.zeros(nchunks, bool)
    seen = set()
    for j in range(nchunks):
        w = int(chunk_win[j])
        if w not in seen:
            first_of_win[j] = True
            seen.add(w)
    seen = set()
    for j in range(nchunks - 1, -1, -1):
        w = int(chunk_win[j])
        if w not in seen:
            last_of_win[j] = True
            seen.add(w)

    # first destination chunk per (w, b)
    base_by_wb = {}
    for j in range(nchunks):
        key2 = (int(chunk_win[j]), int(chunk_bucket[j]))
        if key2 not in base_by_wb:
            base_by_wb[key2] = j

    # segment boundaries of (core, window, bucket) runs in the sorted list
    seg_lo_idx = np.searchsorted(
        cwb_sorted, np.arange(n_cores * nwin * nbuck), side="left"
    )
    seg_hi_idx = np.searchsorted(
        cwb_sorted, np.arange(n_cores * nwin * nbuck), side="right"
    )

    src_cores = []
    tloc_cores = []
    dinv_cores = []
    gcnt_cores = []
    for c in range(n_cores):
        src_flat = np.full(nchunks * P, -1, np.int16)
        tloc_flat = np.full(nchunks * P, -1.0, np.float32)
        for w in range(nwin):
            for b in range(nbuck):
                if (w, b) not in base_by_wb:
                    continue
                s = c * nwin * nbuck + w * nbuck + b
                i0, i1 = seg_lo_idx[s], seg_hi_idx[s]
                m = i1 - i0
                if m == 0:
                    continue
                d0 = base_by_wb[(w, b)] * P
                src_flat[d0 : d0 + m] = rs[i0:i1].astype(np.int16)
                tloc_flat[d0 : d0 + m] = (cs[i0:i1] - c * npc - w * P).astype(
                    np.float32
                )
        # per-core true (non-negative) index count per call; the padded tail
        # of each call is all -1 and generates no descriptors.  An
        # all-negative call would break the DGE, so force >= 1 real idx.
        gcnt = np.zeros(len(call_sizes), np.int32)
        for ci, ((w, b, q0), r) in enumerate(zip(call_meta, call_sizes)):
            m = int(cnt[c, w, b])
            real = min(max(m - q0 * P, 0), r * P)
            if real == 0:
                s0 = (base_by_wb[(w, b)] + q0) * P
                src_flat[s0] = 0
                real = 1
            gcnt[ci] = real
        gcnt_cores.append(np.ascontiguousarray(gcnt[None, :]))

        # dma_gather idx layout: idx i -> partition i%16, col i//16,
        # replicated over the 8 groups of 16 partitions.
        a = src_flat.reshape(nchunks, 8, 16)  # [j, p//16, p%16]
        a = np.transpose(a, (2, 0, 1)).reshape(16, nchunks * 8)
        src_cores.append(np.ascontiguousarray(np.tile(a, (8, 1))))
        tloc_cores.append(
            np.ascontiguousarray(tloc_flat.reshape(nchunks, P).T.astype(BF16))
        )

        dv = np.zeros(nwin * P, np.float32)
        dv[:npc] = dinv_all[c * npc : (c + 1) * npc]
        dinv_cores.append(np.ascontiguousarray(dv.reshape(nwin, P).T))

    return dict(
        npc=npc,
        nwin=nwin,
        nbuck=nbuck,
        spb=spb,
        bucket_rows=bucket_rows,
        nchunks=nchunks,
        chunk_win=chunk_win,
        chunk_bucket=chunk_bucket,
        call_sizes=call_sizes,
        first_of_win=first_of_win,
        last_of_win=last_of_win,
        src_cores=src_cores,
        tloc_cores=tloc_cores,
        dinv_cores=dinv_cores,
        gcnt_cores=gcnt_cores,
    )


def _build_nc(
    n,
    nfeat,
    nhid,
    n_cores,
    nwin,
    nbuck,
    bucket_rows,
    spb,
    nchunks,
    chunk_win,
    chunk_bucket,
    call_sizes,
    first_of_win,
    last_of_win,
    alpha,
    has_bias,
    gather_bufs=6,
    slab_wins=8,
    dma_scratch=32768,
):
    npc_pad = nwin * P
    npc = n // n_cores
    assert nfeat % P == 0
    nk = nfeat // P  # contraction tiles for x @ W

    nc = bacc.Bacc(
        "TRN2",
        target_bir_lowering=False,
        debug=False,
        enable_asserts=False,
        num_devices=n_cores,
        num_swdge_queues=4,
        dynamic_dma_scratch_size=dma_scratch,
    )

    # x staged pre-transposed+interleaved on host: x_in[p, k, m] = x[m, k*P+p]
    x_in = nc.dram_tensor("x_sh", [P, nk, npc_pad], BF16_T, kind="ExternalInput")
    w_in = nc.dram_tensor("w_sn", [nfeat, nhid], BF16_T, kind="ExternalInput")
    dinv_in = nc.dram_tensor("dinv", [P, nwin], F32, kind="ExternalInput")
    bias_in = nc.dram_tensor("bias_t", [P, nhid], F32, kind="ExternalInput")
    max_call = max(call_sizes)
    iota_in = nc.dram_tensor("iota_t", [P, max_call * P], BF16_T, kind="ExternalInput")
    src_in = nc.dram_tensor("src_idx", [P, nchunks * 8], I16, kind="ExternalInput")
    tloc_in = nc.dram_tensor("tloc", [P, nchunks], BF16_T, kind="ExternalInput")
    gcnt_in = nc.dram_tensor(
        "gcnt", [1, len(call_sizes)], mybir.dt.int32, kind="ExternalInput"
    )
    out_d = nc.dram_tensor("out_sh", [npc_pad, nhid], F32, kind="ExternalOutput")

    assert sum(call_sizes) == nchunks

    with tile.TileContext(nc) as tc:
        with (
            tc.tile_pool(name="consts", bufs=1) as cpool,
            tc.tile_pool(name="dram", bufs=1, space="DRAM") as dpool,
        ):
            # constants
            w_sb = cpool.tile([P, nk, nhid], BF16_T)
            nc.sync.dma_start(w_sb[:], w_in[:].rearrange("(k p) h -> p k h", p=P))
            bias_sb = cpool.tile([P, nhid], F32)
            nc.sync.dma_start(bias_sb[:], bias_in[:])
            iota_sb = cpool.tile([P, max_call * P], BF16_T)
            nc.sync.dma_start(iota_sb[:], iota_in[:])
            dinv_sb = cpool.tile([P, nwin], F32)
            nc.sync.dma_start(dinv_sb[:], dinv_in[:])
            ident = cpool.tile([P, P], BF16_T)
            make_identity(nc, ident[:])
            src_sb = cpool.tile([P, nchunks * 8], I16)
            nc.sync.dma_start(src_sb[:], src_in[:])
            tloc_sb = cpool.tile([P, nchunks], BF16_T)
            nc.sync.dma_start(tloc_sb[:], tloc_in[:])
            gcnt_sb = cpool.tile([1, len(call_sizes)], mybir.dt.int32)
            nc.sync.dma_start(gcnt_sb[:], gcnt_in[:])
            gcnt_reg = nc.gpsimd.alloc_register("gcnt_reg")

            # phase-1 output kept resident in SBUF for the self-loop matmuls
            xwp_sb = cpool.tile([P, nwin * nhid], BF16_T)

            ag_in = dpool.tile([npc, nhid], BF16_T)
            ag_out = dpool.tile([n, nhid], BF16_T, addr_space="Shared")

            # ---- phase 1: xw' = dinv[s] * (x_s @ W_sn) for owned nodes ----
            with (
                tc.tile_pool(name="p1x", bufs=3) as xpool,
                tc.tile_pool(name="p1pm", bufs=4, space="PSUM") as psumXW,
            ):
                for s0 in range(0, nwin, slab_wins):
                    ns = min(slab_wins, nwin - s0)
                    xt = xpool.tile([P, nk, slab_wins * P], BF16_T)
                    nc.sync.dma_start(
                        xt[:, :, : ns * P],
                        x_in[:, :, s0 * P : (s0 + ns) * P],
                    )
                    for wr in range(ns):
                        w = s0 + wr
                        nrow = min(P, npc - w * P)
                        pxw = psumXW.tile([P, nhid], F32)
                        for k in range(nk):
                            nc.tensor.matmul(
                                pxw[:],
                                lhsT=xt[:, k, wr * P : (wr + 1) * P],
                                rhs=w_sb[:, k, :],
                                start=(k == 0),
                                stop=(k == nk - 1),
                            )
                        seg = xwp_sb[:, w * nhid : (w + 1) * nhid]
                        nc.scalar.activation(
                            out=seg,
                            in_=pxw[:],
                            func=mybir.ActivationFunctionType.Copy,
                            scale=dinv_sb[:, w : w + 1],
                        )
                        nc.sync.dma_start(
                            ag_in[w * P : w * P + nrow, :], seg[:nrow]
                        )

            nc.gpsimd.collective_compute(
                "AllGather",
                mybir.AluOpType.bypass,
                replica_groups=[list(range(n_cores))],
                ins=[ag_in[:]],
                outs=[ag_out[:]],
            )

            # ---- phase 2: gather + one-hot matmul scatter-add + epilogue ----
            psum_by_win = {}
            with (
                tc.tile_pool(name="gat", bufs=gather_bufs) as gpool,
                tc.tile_pool(name="sel", bufs=6) as spool,
                tc.tile_pool(name="og", bufs=4) as opool,
                tc.tile_pool(name="acc", bufs=8, space="PSUM") as ppool,
            ):
                j = 0
                for ci, r in enumerate(call_sizes):
                    gbuf = gpool.tile(
                        [P, max_call * nhid], BF16_T, tag="gbuf", name="gbuf"
                    )
                    if ci < gather_bufs:
                        # skipped (negative-idx) tail slots are never written;
                        # zero once so stale SBUF can't inject NaN into matmuls
                        nc.vector.memset(gbuf[:], 0.0)
                    b = int(chunk_bucket[j])
                    rows = min(bucket_rows, n - b * bucket_rows)
                    nc.gpsimd.reg_load(gcnt_reg, gcnt_sb[0:1, ci : ci + 1])
                    nc.gpsimd.dma_gather(
                        gbuf[:, : r * nhid].rearrange("p (k e) -> p k e", e=nhid),
                        ag_out[b * bucket_rows : b * bucket_rows + rows, :],
                        src_sb[:, j * 8 : (j + r) * 8],
                        r * P,
                        gcnt_reg,
                        nhid,
                        queue_num=ci % 4,
                    )
                    # one-hot selectors for the whole call in one DVE op
                    sel_big = spool.tile(
                        [P, max_call * P], BF16_T, tag="sel", name="sel_big"
                    )
                    nc.vector.tensor_tensor(
                        out=sel_big[:, : r * P].rearrange("p (k e) -> p k e", e=P),
                        in0=tloc_sb[:, j : j + r].to_broadcast([P, r, P]),
                        in1=iota_sb[:, : r * P].rearrange("p (k e) -> p k e", e=P),
                        op=mybir.AluOpType.is_equal,
                    )
                    for kk in range(r):
                        w = int(chunk_win[j])
                        if first_of_win[j]:
                            pw = ppool.tile([P, nhid], F32, tag="pw", name="pw")
                            psum_by_win[w] = pw
                            # self-loop: identity selector over the resident
                            # phase-1 tile (start=True resets the bank)
                            nc.tensor.matmul(
                                pw[:],
                                lhsT=ident[:],
                                rhs=xwp_sb[:, w * nhid : (w + 1) * nhid],
                                start=True,
                                stop=False,
                            )
                        pw = psum_by_win[w]
                        nc.tensor.matmul(
                            pw[:],
                            lhsT=sel_big[:, kk * P : (kk + 1) * P],
                            rhs=gbuf[:, kk * nhid : (kk + 1) * nhid],
                            start=False,
                            stop=bool(last_of_win[j]),
                        )
                        if last_of_win[j]:
                            og = opool.tile([P, nhid], F32, tag="og", name="og")
                            nrow = min(P, npc - w * P)
                            if has_bias:
                                nc.scalar.activation(
                                    out=og[:],
                                    in_=pw[:],
                                    func=mybir.ActivationFunctionType.Copy,
                                    scale=dinv_sb[:, w : w + 1],
                                )
                                nc.vector.tensor_tensor(
                                    out=og[:],
                                    in0=og[:],
                                    in1=bias_sb[:],
                                    op=mybir.AluOpType.add,
                                )
                                t2 = opool.tile(
                                    [P, nhid], F32, tag="t2", name="t2"
                                )
                                nc.vector.tensor_scalar(
                                    out=t2[:],
                                    in0=og[:],
                                    scalar1=0.0,
                                    scalar2=float(alpha),
                                    op0=mybir.AluOpType.min,
                                    op1=mybir.AluOpType.mult,
                                )
                                nc.vector.tensor_scalar_max(og[:], og[:], 0.0)
                                nc.vector.tensor_tensor(
                                    out=og[:],
                                    in0=og[:],
                                    in1=t2[:],
                                    op=mybir.AluOpType.add,
                                )
                            else:
                                # out = PReLU(dinv[t] * agg), one ACT op
                                nc.scalar.activation(
                                    out=og[:],
                                    in_=pw[:],
                                    func=mybir.ActivationFunctionType.Prelu,
                                    scale=dinv_sb[:, w : w + 1],
                                    alpha=float(alpha),
                                )
                            nc.sync.dma_start(
                                out_d[w * P : w * P + nrow, :], og[:nrow]
                            )
                        j += 1

    nc.compile()
    return nc


def kernel(**inputs):
    x = np.asarray(inputs["x"], dtype=np.float32)
    edge_index = np.asarray(inputs["edge_index"])
    W = np.asarray(inputs["W"], dtype=np.float32)
    bias = np.asarray(inputs["bias"], dtype=np.float32)
    prelu_a = np.asarray(inputs["prelu_a"], dtype=np.float32)
    u = np.asarray(inputs["u"], dtype=np.float32)

    n, nfeat = x.shape
    nhid = W.shape[1]
    n_cores = 8
    win_group = 4
    nbuck = -(-n // 32767)  # int16 index reach per dma_gather bucket
    alpha = float(prelu_a.reshape(-1)[0])
    has_bias = bool(np.any(bias != 0.0))

    # one dma_gather call must stay under the SWDGE ring carveout
    # (dynamic_dma_scratch_size//16 descriptors); 7 chunks = 896 < 1024
    max_call_chunks = 7
    dma_scratch = 16384

    npc = n // n_cores
    nwin = -(-npc // P)
    npc_pad = nwin * P
    nk = nfeat // P

    w_sn = _spectral_norm_host(W, u)
    prep = _prep_host(n, edge_index, n_cores, win_group, nbuck, max_call_chunks)
    nchunks = prep["nchunks"]

    nc = _build_nc(
        n,
        nfeat,
        nhid,
        n_cores,
        nwin,
        prep["nbuck"],
        prep["bucket_rows"],
        prep["spb"],
        nchunks,
        prep["chunk_win"],
        prep["chunk_bucket"],
        prep["call_sizes"],
        prep["first_of_win"],
        prep["last_of_win"],
        alpha,
        has_bias,
        dma_scratch=dma_scratch,
    )

    bias_t = np.ascontiguousarray(np.tile(bias[None, :], (P, 1)))
    max_call = max(prep["call_sizes"])
    iota_t = np.ascontiguousarray(
        np.tile(
            np.tile(np.arange(P, dtype=np.float32), max_call)[None, :], (P, 1)
        ).astype(BF16)
    )
    w_bf = np.ascontiguousarray(w_sn.astype(BF16))

    in_maps = []
    for c in range(n_cores):
        xp = np.zeros((npc_pad, nfeat), np.float32)
        xp[:npc] = x[c * npc : (c + 1) * npc]
        # x_sh[p, k, m] = xp[m, k*P + p]
        x_sh = np.ascontiguousarray(
            xp.T.reshape(nk, P, npc_pad).transpose(1, 0, 2).astype(BF16)
        )
        in_maps.append(
            {
                "x_sh": x_sh,
                "w_sn": w_bf,
                "dinv": prep["dinv_cores"][c],
                "bias_t": bias_t,
                "iota_t": iota_t,
                "src_idx": prep["src_cores"][c],
                "tloc": prep["tloc_cores"][c],
                "gcnt": prep["gcnt_cores"][c],
            }
        )

    res = run_bass_kernel_spmd(
        nc, in_maps, core_ids=list(range(n_cores)), trace=TRACE
    )
    global LAST_RESULT
    LAST_RESULT = res
    out = np.concatenate(
        [res.results[c]["out_sh"][:npc] for c in range(n_cores)], axis=0
    )
    return out


# revision 36
# speedup vs baseline: 1.3934x; 1.0298x over previous
# GCN encoder (DGI) forward on 8 Trainium2 NeuronCores.
#
# Node-partitioned (graph-parallel) sharding, bf16 message table:
#   - nodes are split contiguously across the 8 cores (N/8 per core)
#   - each core owns the edges whose *target* lands in its node range
#   - phase 1: every core computes xw' = dinv[s] * (x_s @ W_sn) in bf16 for
#     its own nodes (x is staged pre-transposed in bf16 so the matmul needs
#     no PE transposes), then an AllGather replicates the bf16 xw' table
#   - phase 2: each core gathers source rows for its edges with bulk
#     indirect DMA (256B bf16 rows), scatter-adds them into per-window PSUM
#     accumulators with one-hot selector matmuls on the PE (selectors built
#     in bf16 on DVE), folds the self-loop in as an identity-selector matmul
#     on the SBUF-resident phase-1 tiles, and runs the whole epilogue
#     (dinv[t] scale + PReLU) as a single ACT op per window.
#
# Host-side work is limited to index preprocessing (edge routing/sorting,
# degree counting, layout shuffles) and the tiny spectral-norm power
# iteration on W.

import numpy as np

import concourse.bacc as bacc
import concourse.bass as bass
import concourse.mybir as mybir
import concourse.tile as tile
from concourse.bass_utils import run_bass_kernel_spmd
from concourse.masks import make_identity

try:
    import ml_dtypes

    BF16 = np.dtype(ml_dtypes.bfloat16)
except ImportError:  # pragma: no cover
    BF16 = None

P = 128
F32 = mybir.dt.float32
BF16_T = mybir.dt.bfloat16
I16 = mybir.dt.int16

# test-harness hooks (ignored in grading): set TRACE=True before calling
# kernel() to capture an NTFF profile; the BassKernelResults lands in
# LAST_RESULT.
TRACE = False
LAST_RESULT = None


def _l2n(v, eps=1e-12):
    return v / (np.linalg.norm(v) + eps)


def _spectral_norm_host(W, u):
    W = W.astype(np.float32)
    u = u.astype(np.float32)
    v = _l2n(W.T @ u)
    u2 = _l2n(W @ v)
    sigma = np.float32(u2 @ (W @ v))
    return W / sigma


def _prep_host(n, edge_index, n_cores, win_group, nbuck, max_call_chunks):
    """Route edges to cores by target and build the SPMD chunk schedule.

    Chunks are 128 edges, each mapping into one 128-target window and one
    source bucket (dma_gather has int16 indices, so the gathered table is
    addressed in buckets of `bucket_rows` rows).  The table in DRAM is laid
    out bucket-major: bucket j holds, for every core c, the rows of c's
    nodes whose local id is in [j*spb, (j+1)*spb) — so bucket j is exactly
    the output of the j-th chunked AllGather and gathers on bucket j can
    start as soon as AG_j lands.  Chunk order: for each super-group of
    `win_group` windows, for each bucket, the chunks of the group's
    windows.  One dma_gather call covers one (group, bucket) run.
    Self-loops are NOT in the edge stream (folded in as identity-selector
    matmuls on the device).
    """
    assert n % n_cores == 0
    npc = n // n_cores
    nwin = -(-npc // P)
    assert npc % nbuck == 0
    spb = npc // nbuck  # rows each core contributes to one bucket
    bucket_rows = spb * n_cores
    assert bucket_rows < 32768

    row = np.ascontiguousarray(edge_index[0]).astype(np.int64)
    col = np.ascontiguousarray(edge_index[1]).astype(np.int64)

    # bucket-major table layout (bucket j = output of chunked AllGather j):
    # node s lives in bucket (s%npc)//spb at row (s//npc)*spb + s%spb
    sbuck = (row % npc) // spb
    srow = (row // npc) * spb + (row % spb)

    # sort all edges by (target window, source bucket) so each (core, window,
    # bucket) run is contiguous; target order within a chunk is free (tloc).
    wkey = (col // npc) * nwin + (col % npc) // P  # global window id
    key = wkey * nbuck + sbuck
    order = np.argsort(key, kind="stable")
    rs = srow[order]
    cs = col[order]
    cwb_sorted = key[order]

    deg = 1.0 + np.bincount(col, minlength=n).astype(np.float64)  # + self loop
    dinv_all = (deg ** -0.5).astype(np.float32)

    # counts per (core, window, bucket)
    cnt = np.bincount(key, minlength=n_cores * nwin * nbuck).reshape(
        n_cores, nwin, nbuck
    )
    kwb = -(-cnt // P)  # chunks per (c, w, b)
    kwb = kwb.max(axis=0)  # [nwin, nbuck] shared schedule

    # chunk order + gather-call runs.  Each call covers chunks of ONE
    # (window, bucket) cell so that every call's padded tail is a run of
    # negative indices; with num_idxs_reg = the per-core true count the DGE
    # skips the tail entirely (no descriptors, no bytes).
    chunk_win = []
    chunk_bucket = []
    call_sizes = []  # chunks per dma_gather call
    call_meta = []  # (w, b, q0) chunk offset of the call within its cell
    for wg in range(0, nwin, win_group):
        ws = range(wg, min(wg + win_group, nwin))
        for b in range(nbuck):
            for w in ws:
                r = int(kwb[w, b])
                if r == 0:
                    continue
                q0 = 0
                while q0 < r:
                    call_sizes.append(min(r - q0, max_call_chunks))
                    call_meta.append((w, b, q0))
                    q0 += max_call_chunks
                chunk_win.extend([w] * r)
                chunk_bucket.extend([b] * r)
    chunk_win = np.asarray(chunk_win)
    chunk_bucket = np.asarray(chunk_bucket)
    nchunks = len(chunk_win)

    # first/last chunk per window in this order
    first_of_win = np.zeros(nchunks, bool)
    last_of_win = np.zeros(nchunks, bool)
    seen = set()
    for j in range(nchunks):
        w = int(chunk_win[j])
        if w not in seen:
            first_of_win[j] = True
            seen.add(w)
    seen = set()
    for j in range(nchunks - 1, -1, -1):
        w = int(chunk_win[j])
        if w not in seen:
            last_of_win[j] = True
            seen.add(w)

    # first destination chunk per (w, b)
    base_by_wb = {}
    for j in range(nchunks):
        key2 = (int(chunk_win[j]), int(chunk_bucket[j]))
        if key2 not in base_by_wb:
            base_by_wb[key2] = j

    # segment boundaries of (core, window, bucket) runs in the sorted list
    seg_lo_idx = np.searchsorted(
        cwb_sorted, np.arange(n_cores * nwin * nbuck), side="left"
    )
    seg_hi_idx = np.searchsorted(
        cwb_sorted, np.arange(n_cores * nwin * nbuck), side="right"
    )

    src_cores = []
    tloc_cores = []
    dinv_cores = []
    gcnt_cores = []
    for c in range(n_cores):
        src_flat = np.full(nchunks * P, -1, np.int16)
        tloc_flat = np.full(nchunks * P, -1.0, np.float32)
        for w in range(nwin):
            for b in range(nbuck):
                if (w, b) not in base_by_wb:
                    continue
                s = c * nwin * nbuck + w * nbuck + b
                i0, i1 = seg_lo_idx[s], seg_hi_idx[s]
                m = i1 - i0
                if m == 0:
                    continue
                d0 = base_by_wb[(w, b)] * P
                src_flat[d0 : d0 + m] = rs[i0:i1].astype(np.int16)
                tloc_flat[d0 : d0 + m] = (cs[i0:i1] - c * npc - w * P).astype(
                    np.float32
                )
        # per-core true (non-negative) index count per call; the padded tail
        # of each call is all -1 and generates no descriptors.  An
        # all-negative call would break the DGE, so force >= 1 real idx.
        gcnt = np.zeros(len(call_sizes), np.int32)
        for ci, ((w, b, q0), r) in enumerate(zip(call_meta, call_sizes)):
            m = int(cnt[c, w, b])
            real = min(max(m - q0 * P, 0), r * P)
            if real == 0:
                s0 = (base_by_wb[(w, b)] + q0) * P
                src_flat[s0] = 0
                real = 1
            gcnt[ci] = real
        gcnt_cores.append(np.ascontiguousarray(gcnt[None, :]))

        # dma_gather idx layout: idx i -> partition i%16, col i//16,
        # replicated over the 8 groups of 16 partitions.
        a = src_flat.reshape(nchunks, 8, 16)  # [j, p//16, p%16]
        a = np.transpose(a, (2, 0, 1)).reshape(16, nchunks * 8)
        src_cores.append(np.ascontiguousarray(np.tile(a, (8, 1))))
        tloc_cores.append(
            np.ascontiguousarray(tloc_flat.reshape(nchunks, P).T.astype(BF16))
        )

        dv = np.zeros(nwin * P, np.float32)
        dv[:npc] = dinv_all[c * npc : (c + 1) * npc]
        dinv_cores.append(np.ascontiguousarray(dv.reshape(nwin, P).T))

    return dict(
        npc=npc,
        nwin=nwin,
        nbuck=nbuck,
        spb=spb,
        bucket_rows=bucket_rows,
        nchunks=nchunks,
        chunk_win=chunk_win,
        chunk_bucket=chunk_bucket,
        call_sizes=call_sizes,
        first_of_win=first_of_win,
        last_of_win=last_of_win,
        src_cores=src_cores,
        tloc_cores=tloc_cores,
        dinv_cores=dinv_cores,
        gcnt_cores=gcnt_cores,
    )


def _build_nc(
    n,
    nfeat,
    nhid,
    n_cores,
    nwin,
    nbuck,
    bucket_rows,
    spb,
    nchunks,
    chunk_win,
    chunk_bucket,
    call_sizes,
    first_of_win,
    last_of_win,
    alpha,
    has_bias,
    gather_bufs=6,
    slab_wins=8,
    dma_scratch=32768,
):
    npc_pad = nwin * P
    npc = n // n_cores
    assert nfeat % P == 0
    nk = nfeat // P  # contraction tiles for x @ W

    nc = bacc.Bacc(
        "TRN2",
        target_bir_lowering=False,
        debug=False,
        enable_asserts=False,
        num_devices=n_cores,
        num_swdge_queues=4,
        dynamic_dma_scratch_size=dma_scratch,
    )

    # x staged pre-transposed+interleaved on host: x_in[p, k, m] = x[m, k*P+p]
    x_in = nc.dram_tensor("x_sh", [P, nk, npc_pad], BF16_T, kind="ExternalInput")
    w_in = nc.dram_tensor("w_sn", [nfeat, nhid], BF16_T, kind="ExternalInput")
    dinv_in = nc.dram_tensor("dinv", [P, nwin], F32, kind="ExternalInput")
    bias_in = nc.dram_tensor("bias_t", [P, nhid], F32, kind="ExternalInput")
    max_call = max(call_sizes)
    iota_in = nc.dram_tensor("iota_t", [P, max_call * P], BF16_T, kind="ExternalInput")
    src_in = nc.dram_tensor("src_idx", [P, nchunks * 8], I16, kind="ExternalInput")
    tloc_in = nc.dram_tensor("tloc", [P, nchunks], BF16_T, kind="ExternalInput")
    gcnt_in = nc.dram_tensor(
        "gcnt", [1, len(call_sizes)], mybir.dt.int32, kind="ExternalInput"
    )
    out_d = nc.dram_tensor("out_sh", [npc_pad, nhid], F32, kind="ExternalOutput")

    assert sum(call_sizes) == nchunks

    with tile.TileContext(nc) as tc:
        with (
            tc.tile_pool(name="consts", bufs=1) as cpool,
            tc.tile_pool(name="dram", bufs=1, space="DRAM") as dpool,
        ):
            # constants
            w_sb = cpool.tile([P, nk, nhid], BF16_T)
            nc.sync.dma_start(w_sb[:], w_in[:].rearrange("(k p) h -> p k h", p=P))
            bias_sb = cpool.tile([P, nhid], F32)
            nc.sync.dma_start(bias_sb[:], bias_in[:])
            iota_sb = cpool.tile([P, max_call * P], BF16_T)
            nc.sync.dma_start(iota_sb[:], iota_in[:])
            dinv_sb = cpool.tile([P, nwin], F32)
            nc.sync.dma_start(dinv_sb[:], dinv_in[:])
            ident = cpool.tile([P, P], BF16_T)
            make_identity(nc, ident[:])
            src_sb = cpool.tile([P, nchunks * 8], I16)
            nc.sync.dma_start(src_sb[:], src_in[:])
            tloc_sb = cpool.tile([P, nchunks], BF16_T)
            nc.sync.dma_start(tloc_sb[:], tloc_in[:])
            gcnt_sb = cpool.tile([1, len(call_sizes)], mybir.dt.int32)
            nc.sync.dma_start(gcnt_sb[:], gcnt_in[:])
            KREG = 8
            gcnt_regs = [
                nc.gpsimd.alloc_register(f"gcnt_reg{i}") for i in range(KREG)
            ]

            # phase-1 output kept resident in SBUF for the self-loop matmuls
            xwp_sb = cpool.tile([P, nwin * nhid], BF16_T)

            ag_in = dpool.tile([npc, nhid], BF16_T)
            ag_out = [
                dpool.tile(
                    [bucket_rows, nhid],
                    BF16_T,
                    addr_space="Shared",
                    name=f"ag_out{j}",
                )
                for j in range(nbuck)
            ]

            # ---- phase 1: xw' = dinv[s] * (x_s @ W_sn) for owned nodes ----
            # The AllGather is chunked so AG_1..3 overlap the tail of phase 1.
            # AG_0 (the gate for the first gather calls, which are bucket 0)
            # is emitted LAST so no gather overlaps a running collective —
            # concurrent gather+collective DMA trips the HW DMA throttle.
            def _fire_ag(j):
                nc.gpsimd.collective_compute(
                    "AllGather",
                    mybir.AluOpType.bypass,
                    replica_groups=[list(range(n_cores))],
                    ins=[ag_in[j * spb : (j + 1) * spb]],
                    outs=[ag_out[j][:]],
                )

            ag_trigger = {(-(-spb * (j + 1) // P)) - 1: j for j in range(1, nbuck)}
            with (
                tc.tile_pool(name="p1x", bufs=3) as xpool,
                tc.tile_pool(name="p1pm", bufs=4, space="PSUM") as psumXW,
            ):
                for s0 in range(0, nwin, slab_wins):
                    ns = min(slab_wins, nwin - s0)
                    xt = xpool.tile([P, nk, slab_wins * P], BF16_T)
                    nc.sync.dma_start(
                        xt[:, :, : ns * P],
                        x_in[:, :, s0 * P : (s0 + ns) * P],
                    )
                    for wr in range(ns):
                        w = s0 + wr
                        nrow = min(P, npc - w * P)
                        pxw = psumXW.tile([P, nhid], F32)
                        for k in range(nk):
                            nc.tensor.matmul(
                                pxw[:],
                                lhsT=xt[:, k, wr * P : (wr + 1) * P],
                                rhs=w_sb[:, k, :],
                                start=(k == 0),
                                stop=(k == nk - 1),
                            )
                        seg = xwp_sb[:, w * nhid : (w + 1) * nhid]
                        nc.scalar.activation(
                            out=seg,
                            in_=pxw[:],
                            func=mybir.ActivationFunctionType.Copy,
                            scale=dinv_sb[:, w : w + 1],
                        )
                        nc.sync.dma_start(
                            ag_in[w * P : w * P + nrow, :], seg[:nrow]
                        )
                        if w in ag_trigger:
                            _fire_ag(ag_trigger[w])

            _fire_ag(0)

            # ---- phase 2: gather + one-hot matmul scatter-add + epilogue ----
            psum_by_win = {}
            with (
                tc.tile_pool(name="gat", bufs=gather_bufs) as gpool,
                tc.tile_pool(name="sel", bufs=6) as spool,
                tc.tile_pool(name="og", bufs=4) as opool,
                tc.tile_pool(name="acc", bufs=8, space="PSUM") as ppool,
            ):
                j = 0
                for ci, r in enumerate(call_sizes):
                    gbuf = gpool.tile(
                        [P, max_call * nhid], BF16_T, tag="gbuf", name="gbuf"
                    )
                    if ci < gather_bufs:
                        # skipped (negative-idx) tail slots are never written;
                        # zero once so stale SBUF can't inject NaN into matmuls
                        nc.vector.memset(gbuf[:], 0.0)
                    b = int(chunk_bucket[j])
                    if ci % KREG == 0:
                        nk2 = min(KREG, len(call_sizes) - ci)
                        nc.gpsimd.reg_load(
                            gcnt_regs[:nk2], gcnt_sb[0:1, ci : ci + nk2]
                        )
                    nc.gpsimd.dma_gather(
                        gbuf[:, : r * nhid].rearrange("p (k e) -> p k e", e=nhid),
                        ag_out[b][:],
                        src_sb[:, j * 8 : (j + r) * 8],
                        r * P,
                        gcnt_regs[ci % KREG],
                        nhid,
                        queue_num=ci % 4,
                    )
                    # one-hot selectors for the whole call in one DVE op
                    sel_big = spool.tile(
                        [P, max_call * P], BF16_T, tag="sel", name="sel_big"
                    )
                    nc.vector.tensor_tensor(
                        out=sel_big[:, : r * P].rearrange("p (k e) -> p k e", e=P),
                        in0=tloc_sb[:, j : j + r].to_broadcast([P, r, P]),
                        in1=iota_sb[:, : r * P].rearrange("p (k e) -> p k e", e=P),
                        op=mybir.AluOpType.is_equal,
                    )
                    for kk in range(r):
                        w = int(chunk_win[j])
                        if first_of_win[j]:
                            pw = ppool.tile([P, nhid], F32, tag="pw", name="pw")
                            psum_by_win[w] = pw
                            # self-loop: identity selector over the resident
                            # phase-1 tile (start=True resets the bank)
                            nc.tensor.matmul(
                                pw[:],
                                lhsT=ident[:],
                                rhs=xwp_sb[:, w * nhid : (w + 1) * nhid],
                                start=True,
                                stop=False,
                            )
                        pw = psum_by_win[w]
                        nc.tensor.matmul(
                            pw[:],
                            lhsT=sel_big[:, kk * P : (kk + 1) * P],
                            rhs=gbuf[:, kk * nhid : (kk + 1) * nhid],
                            start=False,
                            stop=bool(last_of_win[j]),
                        )
                        if last_of_win[j]:
                            og = opool.tile([P, nhid], F32, tag="og", name="og")
                            nrow = min(P, npc - w * P)
                            if has_bias:
                                nc.scalar.activation(
                                    out=og[:],
                                    in_=pw[:],
                                    func=mybir.ActivationFunctionType.Copy,
                                    scale=dinv_sb[:, w : w + 1],
                                )
                                nc.vector.tensor_tensor(
                                    out=og[:],
                                    in0=og[:],
                                    in1=bias_sb[:],
                                    op=mybir.AluOpType.add,
                                )
                                t2 = opool.tile(
                                    [P, nhid], F32, tag="t2", name="t2"
                                )
                                nc.vector.tensor_scalar(
                                    out=t2[:],
                                    in0=og[:],
                                    scalar1=0.0,
                                    scalar2=float(alpha),
                                    op0=mybir.AluOpType.min,
                                    op1=mybir.AluOpType.mult,
                                )
                                nc.vector.tensor_scalar_max(og[:], og[:], 0.0)
                                nc.vector.tensor_tensor(
                                    out=og[:],
                                    in0=og[:],
                                    in1=t2[:],
                                    op=mybir.AluOpType.add,
                                )
                            else:
                                # out = PReLU(dinv[t] * agg), one ACT op
                                nc.scalar.activation(
                                    out=og[:],
                                    in_=pw[:],
                                    func=mybir.ActivationFunctionType.Prelu,
                                    scale=dinv_sb[:, w : w + 1],
                                    alpha=float(alpha),
                                )
                            nc.sync.dma_start(
                                out_d[w * P : w * P + nrow, :], og[:nrow]
                            )
                        j += 1

    nc.compile()
    return nc


def kernel(**inputs):
    x = np.asarray(inputs["x"], dtype=np.float32)
    edge_index = np.asarray(inputs["edge_index"])
    W = np.asarray(inputs["W"], dtype=np.float32)
    bias = np.asarray(inputs["bias"], dtype=np.float32)
    prelu_a = np.asarray(inputs["prelu_a"], dtype=np.float32)
    u = np.asarray(inputs["u"], dtype=np.float32)

    n, nfeat = x.shape
    nhid = W.shape[1]
    n_cores = 8
    win_group = 4
    nbuck = -(-n // 32767)  # int16 index reach per dma_gather bucket
    alpha = float(prelu_a.reshape(-1)[0])
    has_bias = bool(np.any(bias != 0.0))

    # one dma_gather call must stay under the SWDGE ring carveout
    # (dynamic_dma_scratch_size//16 descriptors); 7 chunks = 896 < 1024
    max_call_chunks = 7
    dma_scratch = 16384

    npc = n // n_cores
    nwin = -(-npc // P)
    npc_pad = nwin * P
    nk = nfeat // P

    w_sn = _spectral_norm_host(W, u)
    prep = _prep_host(n, edge_index, n_cores, win_group, nbuck, max_call_chunks)
    nchunks = prep["nchunks"]

    nc = _build_nc(
        n,
        nfeat,
        nhid,
        n_cores,
        nwin,
        prep["nbuck"],
        prep["bucket_rows"],
        prep["spb"],
        nchunks,
        prep["chunk_win"],
        prep["chunk_bucket"],
        prep["call_sizes"],
        prep["first_of_win"],
        prep["last_of_win"],
        alpha,
        has_bias,
        dma_scratch=dma_scratch,
    )

    bias_t = np.ascontiguousarray(np.tile(bias[None, :], (P, 1)))
    max_call = max(prep["call_sizes"])
    iota_t = np.ascontiguousarray(
        np.tile(
            np.tile(np.arange(P, dtype=np.float32), max_call)[None, :], (P, 1)
        ).astype(BF16)
    )
    w_bf = np.ascontiguousarray(w_sn.astype(BF16))

    in_maps = []
    for c in range(n_cores):
        xp = np.zeros((npc_pad, nfeat), np.float32)
        xp[:npc] = x[c * npc : (c + 1) * npc]
        # x_sh[p, k, m] = xp[m, k*P + p]
        x_sh = np.ascontiguousarray(
            xp.T.reshape(nk, P, npc_pad).transpose(1, 0, 2).astype(BF16)
        )
        in_maps.append(
            {
                "x_sh": x_sh,
                "w_sn": w_bf,
                "dinv": prep["dinv_cores"][c],
                "bias_t": bias_t,
                "iota_t": iota_t,
                "src_idx": prep["src_cores"][c],
                "tloc": prep["tloc_cores"][c],
                "gcnt": prep["gcnt_cores"][c],
            }
        )

    res = run_bass_kernel_spmd(
        nc, in_maps, core_ids=list(range(n_cores)), trace=TRACE
    )
    global LAST_RESULT
    LAST_RESULT = res
    out = np.concatenate(
        [res.results[c]["out_sh"][:npc] for c in range(n_cores)], axis=0
    )
    return out


# revision 38
# speedup vs baseline: 1.4206x; 1.0195x over previous
# GCN encoder (DGI) forward on 8 Trainium2 NeuronCores.
#
# Node-partitioned (graph-parallel) sharding, bf16 message table:
#   - nodes are split contiguously across the 8 cores (N/8 per core)
#   - each core owns the edges whose *target* lands in its node range
#   - phase 1: every core computes xw' = dinv[s] * (x_s @ W_sn) in bf16 for
#     its own nodes (x is staged pre-transposed in bf16 so the matmul needs
#     no PE transposes), then an AllGather replicates the bf16 xw' table
#   - phase 2: each core gathers source rows for its edges with bulk
#     indirect DMA (256B bf16 rows), scatter-adds them into per-window PSUM
#     accumulators with one-hot selector matmuls on the PE (selectors built
#     in bf16 on DVE), folds the self-loop in as an identity-selector matmul
#     on the SBUF-resident phase-1 tiles, and runs the whole epilogue
#     (dinv[t] scale + PReLU) as a single ACT op per window.
#
# Host-side work is limited to index preprocessing (edge routing/sorting,
# degree counting, layout shuffles) and the tiny spectral-norm power
# iteration on W.

import numpy as np

import concourse.bacc as bacc
import concourse.bass as bass
import concourse.mybir as mybir
import concourse.tile as tile
from concourse.bass_utils import run_bass_kernel_spmd
from concourse.masks import make_identity

try:
    import ml_dtypes

    BF16 = np.dtype(ml_dtypes.bfloat16)
except ImportError:  # pragma: no cover
    BF16 = None

P = 128
F32 = mybir.dt.float32
BF16_T = mybir.dt.bfloat16
I16 = mybir.dt.int16

# test-harness hooks (ignored in grading): set TRACE=True before calling
# kernel() to capture an NTFF profile; the BassKernelResults lands in
# LAST_RESULT.
TRACE = False
LAST_RESULT = None


def _l2n(v, eps=1e-12):
    return v / (np.linalg.norm(v) + eps)


def _spectral_norm_host(W, u):
    W = W.astype(np.float32)
    u = u.astype(np.float32)
    v = _l2n(W.T @ u)
    u2 = _l2n(W @ v)
    sigma = np.float32(u2 @ (W @ v))
    return W / sigma


def _prep_host(n, edge_index, n_cores, win_group, nbuck, max_call_chunks):
    """Route edges to cores by target and build the SPMD chunk schedule.

    Chunks are 128 edges, each mapping into one 128-target window and one
    source bucket (dma_gather has int16 indices, so the gathered table is
    addressed in buckets of `bucket_rows` rows).  The table in DRAM is laid
    out bucket-major: bucket j holds, for every core c, the rows of c's
    nodes whose local id is in [j*spb, (j+1)*spb) — so bucket j is exactly
    the output of the j-th chunked AllGather and gathers on bucket j can
    start as soon as AG_j lands.  Chunk order: for each super-group of
    `win_group` windows, for each bucket, the chunks of the group's
    windows.  One dma_gather call covers one (group, bucket) run.
    Self-loops are NOT in the edge stream (folded in as identity-selector
    matmuls on the device).
    """
    assert n % n_cores == 0
    npc = n // n_cores
    nwin = -(-npc // P)
    assert npc % nbuck == 0
    spb = npc // nbuck  # rows each core contributes to one bucket
    bucket_rows = spb * n_cores
    assert bucket_rows < 32768

    row = np.ascontiguousarray(edge_index[0]).astype(np.int64)
    col = np.ascontiguousarray(edge_index[1]).astype(np.int64)

    # bucket-major table layout (bucket j = output of chunked AllGather j):
    # node s lives in bucket (s%npc)//spb at row (s//npc)*spb + s%spb
    sbuck = (row % npc) // spb
    srow = (row // npc) * spb + (row % spb)

    # sort all edges by (target window, source bucket) so each (core, window,
    # bucket) run is contiguous; target order within a chunk is free (tloc).
    wkey = (col // npc) * nwin + (col % npc) // P  # global window id
    key = wkey * nbuck + sbuck
    order = np.argsort(key, kind="stable")
    rs = srow[order]
    cs = col[order]
    cwb_sorted = key[order]

    deg = 1.0 + np.bincount(col, minlength=n).astype(np.float64)  # + self loop
    dinv_all = (deg ** -0.5).astype(np.float32)

    # counts per (core, window, bucket)
    cnt = np.bincount(key, minlength=n_cores * nwin * nbuck).reshape(
        n_cores, nwin, nbuck
    )
    kwb = -(-cnt // P)  # chunks per (c, w, b)
    kwb = kwb.max(axis=0)  # [nwin, nbuck] shared schedule

    # chunk order + gather-call runs.  Each call covers chunks of ONE
    # (window, bucket) cell so that every call's padded tail is a run of
    # negative indices; with num_idxs_reg = the per-core true count the DGE
    # skips the tail entirely (no descriptors, no bytes).
    chunk_win = []
    chunk_bucket = []
    call_sizes = []  # chunks per dma_gather call
    call_meta = []  # (w, b, q0) chunk offset of the call within its cell
    for wg in range(0, nwin, win_group):
        ws = range(wg, min(wg + win_group, nwin))
        for b in range(nbuck):
            for w in ws:
                r = int(kwb[w, b])
                if r == 0:
                    continue
                q0 = 0
                while q0 < r:
                    call_sizes.append(min(r - q0, max_call_chunks))
                    call_meta.append((w, b, q0))
                    q0 += max_call_chunks
                chunk_win.extend([w] * r)
                chunk_bucket.extend([b] * r)
    chunk_win = np.asarray(chunk_win)
    chunk_bucket = np.asarray(chunk_bucket)
    nchunks = len(chunk_win)

    # first/last chunk per window in this order
    first_of_win = np.zeros(nchunks, bool)
    last_of_win = np.zeros(nchunks, bool)
    seen = set()
    for j in range(nchunks):
        w = int(chunk_win[j])
        if w not in seen:
            first_of_win[j] = True
            seen.add(w)
    seen = set()
    for j in range(nchunks - 1, -1, -1):
        w = int(chunk_win[j])
        if w not in seen:
            last_of_win[j] = True
            seen.add(w)

    # first destination chunk per (w, b)
    base_by_wb = {}
    for j in range(nchunks):
        key2 = (int(chunk_win[j]), int(chunk_bucket[j]))
        if key2 not in base_by_wb:
            base_by_wb[key2] = j

    # segment boundaries of (core, window, bucket) runs in the sorted list
    seg_lo_idx = np.searchsorted(
        cwb_sorted, np.arange(n_cores * nwin * nbuck), side="left"
    )
    seg_hi_idx = np.searchsorted(
        cwb_sorted, np.arange(n_cores * nwin * nbuck), side="right"
    )

    src_cores = []
    tloc_cores = []
    dinv_cores = []
    gcnt_cores = []
    for c in range(n_cores):
        src_flat = np.full(nchunks * P, -1, np.int16)
        tloc_flat = np.full(nchunks * P, -1.0, np.float32)
        for w in range(nwin):
            for b in range(nbuck):
                if (w, b) not in base_by_wb:
                    continue
                s = c * nwin * nbuck + w * nbuck + b
                i0, i1 = seg_lo_idx[s], seg_hi_idx[s]
                m = i1 - i0
                if m == 0:
                    continue
                d0 = base_by_wb[(w, b)] * P
                src_flat[d0 : d0 + m] = rs[i0:i1].astype(np.int16)
                tloc_flat[d0 : d0 + m] = (cs[i0:i1] - c * npc - w * P).astype(
                    np.float32
                )
        # per-core true (non-negative) index count per call; the padded tail
        # of each call is all -1 and generates no descriptors.  An
        # all-negative call would break the DGE, so force >= 1 real idx.
        gcnt = np.zeros(len(call_sizes), np.int32)
        for ci, ((w, b, q0), r) in enumerate(zip(call_meta, call_sizes)):
            m = int(cnt[c, w, b])
            real = min(max(m - q0 * P, 0), r * P)
            if real == 0:
                s0 = (base_by_wb[(w, b)] + q0) * P
                src_flat[s0] = 0
                real = 1
            gcnt[ci] = real
        gcnt_cores.append(np.ascontiguousarray(gcnt[None, :]))

        # dma_gather idx layout: idx i -> partition i%16, col i//16,
        # replicated over the 8 groups of 16 partitions.
        a = src_flat.reshape(nchunks, 8, 16)  # [j, p//16, p%16]
        a = np.transpose(a, (2, 0, 1)).reshape(16, nchunks * 8)
        src_cores.append(np.ascontiguousarray(np.tile(a, (8, 1))))
        tloc_cores.append(
            np.ascontiguousarray(tloc_flat.reshape(nchunks, P).T.astype(BF16))
        )

        dv = np.zeros(nwin * P, np.float32)
        dv[:npc] = dinv_all[c * npc : (c + 1) * npc]
        dinv_cores.append(np.ascontiguousarray(dv.reshape(nwin, P).T))

    return dict(
        npc=npc,
        nwin=nwin,
        nbuck=nbuck,
        spb=spb,
        bucket_rows=bucket_rows,
        nchunks=nchunks,
        chunk_win=chunk_win,
        chunk_bucket=chunk_bucket,
        call_sizes=call_sizes,
        first_of_win=first_of_win,
        last_of_win=last_of_win,
        src_cores=src_cores,
        tloc_cores=tloc_cores,
        dinv_cores=dinv_cores,
        gcnt_cores=gcnt_cores,
    )


def _build_nc(
    n,
    nfeat,
    nhid,
    n_cores,
    nwin,
    nbuck,
    bucket_rows,
    spb,
    nchunks,
    chunk_win,
    chunk_bucket,
    call_sizes,
    first_of_win,
    last_of_win,
    alpha,
    has_bias,
    gather_bufs=12,
    slab_wins=8,
    dma_scratch=16384,
):
    npc_pad = nwin * P
    npc = n // n_cores
    assert nfeat % P == 0
    nk = nfeat // P  # contraction tiles for x @ W

    nc = bacc.Bacc(
        "TRN2",
        target_bir_lowering=False,
        debug=False,
        enable_asserts=False,
        num_devices=n_cores,
        num_swdge_queues=4,
        dynamic_dma_scratch_size=dma_scratch,
    )

    # x staged pre-transposed+interleaved on host: x_in[p, k, m] = x[m, k*P+p]
    x_in = nc.dram_tensor("x_sh", [P, nk, npc_pad], BF16_T, kind="ExternalInput")
    w_in = nc.dram_tensor("w_sn", [nfeat, nhid], BF16_T, kind="ExternalInput")
    dinv_in = nc.dram_tensor("dinv", [P, nwin], F32, kind="ExternalInput")
    bias_in = nc.dram_tensor("bias_t", [P, nhid], F32, kind="ExternalInput")
    max_call = max(call_sizes)
    iota_in = nc.dram_tensor("iota_t", [P, max_call * P], BF16_T, kind="ExternalInput")
    src_in = nc.dram_tensor("src_idx", [P, nchunks * 8], I16, kind="ExternalInput")
    tloc_in = nc.dram_tensor("tloc", [P, nchunks], BF16_T, kind="ExternalInput")
    gcnt_in = nc.dram_tensor(
        "gcnt", [1, len(call_sizes)], mybir.dt.int32, kind="ExternalInput"
    )
    out_d = nc.dram_tensor("out_sh", [npc_pad, nhid], F32, kind="ExternalOutput")

    assert sum(call_sizes) == nchunks

    with tile.TileContext(nc) as tc:
        with (
            tc.tile_pool(name="consts", bufs=1) as cpool,
            tc.tile_pool(name="dram", bufs=1, space="DRAM") as dpool,
        ):
            # constants
            w_sb = cpool.tile([P, nk, nhid], BF16_T)
            nc.sync.dma_start(w_sb[:], w_in[:].rearrange("(k p) h -> p k h", p=P))
            bias_sb = cpool.tile([P, nhid], F32)
            nc.sync.dma_start(bias_sb[:], bias_in[:])
            iota_sb = cpool.tile([P, max_call * P], BF16_T)
            nc.sync.dma_start(iota_sb[:], iota_in[:])
            dinv_sb = cpool.tile([P, nwin], F32)
            nc.sync.dma_start(dinv_sb[:], dinv_in[:])
            ident = cpool.tile([P, P], BF16_T)
            make_identity(nc, ident[:])
            src_sb = cpool.tile([P, nchunks * 8], I16)
            nc.sync.dma_start(src_sb[:], src_in[:])
            tloc_sb = cpool.tile([P, nchunks], BF16_T)
            nc.sync.dma_start(tloc_sb[:], tloc_in[:])
            gcnt_sb = cpool.tile([1, len(call_sizes)], mybir.dt.int32)
            nc.sync.dma_start(gcnt_sb[:], gcnt_in[:])
            KREG = 8
            gcnt_regs = [
                nc.gpsimd.alloc_register(f"gcnt_reg{i}") for i in range(KREG)
            ]

            # phase-1 output kept resident in SBUF for the self-loop matmuls
            xwp_sb = cpool.tile([P, nwin * nhid], BF16_T)

            ag_in = dpool.tile([npc, nhid], BF16_T)
            ag_out = [
                dpool.tile(
                    [bucket_rows, nhid],
                    BF16_T,
                    addr_space="Shared",
                    name=f"ag_out{j}",
                )
                for j in range(nbuck)
            ]

            # ---- phase 1: xw' = dinv[s] * (x_s @ W_sn) for owned nodes ----
            # The AllGather is chunked so AG_1..3 overlap the tail of phase 1.
            # AG_0 (the gate for the first gather calls, which are bucket 0)
            # is emitted LAST so no gather overlaps a running collective —
            # concurrent gather+collective DMA trips the HW DMA throttle.
            def _fire_ag(j):
                nc.gpsimd.collective_compute(
                    "AllGather",
                    mybir.AluOpType.bypass,
                    replica_groups=[list(range(n_cores))],
                    ins=[ag_in[j * spb : (j + 1) * spb]],
                    outs=[ag_out[j][:]],
                )

            ag_trigger = {(-(-spb * (j + 1) // P)) - 1: j for j in range(1, nbuck)}
            with (
                tc.tile_pool(name="p1x", bufs=3) as xpool,
                tc.tile_pool(name="p1pm", bufs=4, space="PSUM") as psumXW,
            ):
                for s0 in range(0, nwin, slab_wins):
                    ns = min(slab_wins, nwin - s0)
                    xt = xpool.tile([P, nk, slab_wins * P], BF16_T)
                    nc.sync.dma_start(
                        xt[:, :, : ns * P],
                        x_in[:, :, s0 * P : (s0 + ns) * P],
                    )
                    for wr in range(ns):
                        w = s0 + wr
                        nrow = min(P, npc - w * P)
                        pxw = psumXW.tile([P, nhid], F32)
                        for k in range(nk):
                            nc.tensor.matmul(
                                pxw[:],
                                lhsT=xt[:, k, wr * P : (wr + 1) * P],
                                rhs=w_sb[:, k, :],
                                start=(k == 0),
                                stop=(k == nk - 1),
                            )
                        seg = xwp_sb[:, w * nhid : (w + 1) * nhid]
                        nc.scalar.activation(
                            out=seg,
                            in_=pxw[:],
                            func=mybir.ActivationFunctionType.Copy,
                            scale=dinv_sb[:, w : w + 1],
                        )
                        nc.sync.dma_start(
                            ag_in[w * P : w * P + nrow, :], seg[:nrow]
                        )
                        if w in ag_trigger:
                            _fire_ag(ag_trigger[w])

            _fire_ag(0)

            # ---- phase 2: gather + one-hot matmul scatter-add + epilogue ----
            psum_by_win = {}
            with (
                tc.tile_pool(name="gat", bufs=gather_bufs) as gpool,
                tc.tile_pool(name="sel", bufs=12) as spool,
                tc.tile_pool(name="og", bufs=4) as opool,
                tc.tile_pool(name="acc", bufs=8, space="PSUM") as ppool,
            ):
                j = 0
                for ci, r in enumerate(call_sizes):
                    gbuf = gpool.tile(
                        [P, max_call * nhid], BF16_T, tag="gbuf", name="gbuf"
                    )
                    if ci < gather_bufs:
                        # skipped (negative-idx) tail slots are never written;
                        # zero once so stale SBUF can't inject NaN into matmuls
                        nc.vector.memset(gbuf[:], 0.0)
                    b = int(chunk_bucket[j])
                    if ci % KREG == 0:
                        nk2 = min(KREG, len(call_sizes) - ci)
                        nc.gpsimd.reg_load(
                            gcnt_regs[:nk2], gcnt_sb[0:1, ci : ci + nk2]
                        )
                    nc.gpsimd.dma_gather(
                        gbuf[:, : r * nhid].rearrange("p (k e) -> p k e", e=nhid),
                        ag_out[b][:],
                        src_sb[:, j * 8 : (j + r) * 8],
                        r * P,
                        gcnt_regs[ci % KREG],
                        nhid,
                        queue_num=ci % 4,
                    )
                    # one-hot selectors for the whole call in one DVE op
                    sel_big = spool.tile(
                        [P, max_call * P], BF16_T, tag="sel", name="sel_big"
                    )
                    nc.vector.tensor_tensor(
                        out=sel_big[:, : r * P].rearrange("p (k e) -> p k e", e=P),
                        in0=tloc_sb[:, j : j + r].to_broadcast([P, r, P]),
                        in1=iota_sb[:, : r * P].rearrange("p (k e) -> p k e", e=P),
                        op=mybir.AluOpType.is_equal,
                    )
                    for kk in range(r):
                        w = int(chunk_win[j])
                        if first_of_win[j]:
                            pw = ppool.tile([P, nhid], F32, tag="pw", name="pw")
                            psum_by_win[w] = pw
                            # self-loop: identity selector over the resident
                            # phase-1 tile (start=True resets the bank)
                            nc.tensor.matmul(
                                pw[:],
                                lhsT=ident[:],
                                rhs=xwp_sb[:, w * nhid : (w + 1) * nhid],
                                start=True,
                                stop=False,
                            )
                        pw = psum_by_win[w]
                        nc.tensor.matmul(
                            pw[:],
                            lhsT=sel_big[:, kk * P : (kk + 1) * P],
                            rhs=gbuf[:, kk * nhid : (kk + 1) * nhid],
                            start=False,
                            stop=bool(last_of_win[j]),
                        )
                        if last_of_win[j]:
                            og = opool.tile([P, nhid], F32, tag="og", name="og")
                            nrow = min(P, npc - w * P)
                            if has_bias:
                                nc.scalar.activation(
                                    out=og[:],
                                    in_=pw[:],
                                    func=mybir.ActivationFunctionType.Copy,
                                    scale=dinv_sb[:, w : w + 1],
                                )
                                nc.vector.tensor_tensor(
                                    out=og[:],
                                    in0=og[:],
                                    in1=bias_sb[:],
                                    op=mybir.AluOpType.add,
                                )
                                t2 = opool.tile(
                                    [P, nhid], F32, tag="t2", name="t2"
                                )
                                nc.vector.tensor_scalar(
                                    out=t2[:],
                                    in0=og[:],
                                    scalar1=0.0,
                                    scalar2=float(alpha),
                                    op0=mybir.AluOpType.min,
                                    op1=mybir.AluOpType.mult,
                                )
                                nc.vector.tensor_scalar_max(og[:], og[:], 0.0)
                                nc.vector.tensor_tensor(
                                    out=og[:],
                                    in0=og[:],
                                    in1=t2[:],
                                    op=mybir.AluOpType.add,
                                )
                            else:
                                # out = PReLU(dinv[t] * agg), one ACT op
                                nc.scalar.activation(
                                    out=og[:],
                                    in_=pw[:],
                                    func=mybir.ActivationFunctionType.Prelu,
                                    scale=dinv_sb[:, w : w + 1],
                                    alpha=float(alpha),
                                )
                            nc.sync.dma_start(
                                out_d[w * P : w * P + nrow, :], og[:nrow]
                            )
                        j += 1

    nc.compile()
    return nc


def kernel(**inputs):
    x = np.asarray(inputs["x"], dtype=np.float32)
    edge_index = np.asarray(inputs["edge_index"])
    W = np.asarray(inputs["W"], dtype=np.float32)
    bias = np.asarray(inputs["bias"], dtype=np.float32)
    prelu_a = np.asarray(inputs["prelu_a"], dtype=np.float32)
    u = np.asarray(inputs["u"], dtype=np.float32)

    n, nfeat = x.shape
    nhid = W.shape[1]
    n_cores = 8
    win_group = 4
    nbuck = -(-n // 32767)  # int16 index reach per dma_gather bucket
    alpha = float(prelu_a.reshape(-1)[0])
    has_bias = bool(np.any(bias != 0.0))

    # one dma_gather call must stay under the SWDGE ring carveout
    # (dynamic_dma_scratch_size//16 descriptors); 7 chunks = 896 < 1024
    max_call_chunks = 7
    dma_scratch = 16384

    npc = n // n_cores
    nwin = -(-npc // P)
    npc_pad = nwin * P
    nk = nfeat // P

    w_sn = _spectral_norm_host(W, u)
    prep = _prep_host(n, edge_index, n_cores, win_group, nbuck, max_call_chunks)
    nchunks = prep["nchunks"]

    nc = _build_nc(
        n,
        nfeat,
        nhid,
        n_cores,
        nwin,
        prep["nbuck"],
        prep["bucket_rows"],
        prep["spb"],
        nchunks,
        prep["chunk_win"],
        prep["chunk_bucket"],
        prep["call_sizes"],
        prep["first_of_win"],
        prep["last_of_win"],
        alpha,
        has_bias,
        dma_scratch=dma_scratch,
    )

    bias_t = np.ascontiguousarray(np.tile(bias[None, :], (P, 1)))
    max_call = max(prep["call_sizes"])
    iota_t = np.ascontiguousarray(
        np.tile(
            np.tile(np.arange(P, dtype=np.float32), max_call)[None, :], (P, 1)
        ).astype(BF16)
    )
    w_bf = np.ascontiguousarray(w_sn.astype(BF16))

    in_maps = []
    for c in range(n_cores):
        xp = np.zeros((npc_pad, nfeat), np.float32)
        xp[:npc] = x[c * npc : (c + 1) * npc]
        # x_sh[p, k, m] = xp[m, k*P + p]
        x_sh = np.ascontiguousarray(
            xp.T.reshape(nk, P, npc_pad).transpose(1, 0, 2).astype(BF16)
        )
        in_maps.append(
            {
                "x_sh": x_sh,
                "w_sn": w_bf,
                "dinv": prep["dinv_cores"][c],
                "bias_t": bias_t,
                "iota_t": iota_t,
                "src_idx": prep["src_cores"][c],
                "tloc": prep["tloc_cores"][c],
                "gcnt": prep["gcnt_cores"][c],
            }
        )

    res = run_bass_kernel_spmd(
        nc, in_maps, core_ids=list(range(n_cores)), trace=TRACE
    )
    global LAST_RESULT
    LAST_RESULT = res
    out = np.concatenate(
        [res.results[c]["out_sh"][:npc] for c in range(n_cores)], axis=0
    )
    return out


# revision 40
# speedup vs baseline: 1.4558x; 1.0248x over previous
# GCN encoder (DGI) forward on 8 Trainium2 NeuronCores.
#
# Node-partitioned (graph-parallel) sharding, bf16 message table:
#   - nodes are split contiguously across the 8 cores (N/8 per core)
#   - each core owns the edges whose *target* lands in its node range
#   - phase 1: every core computes xw' = dinv[s] * (x_s @ W_sn) in bf16 for
#     its own nodes (x is staged pre-transposed in bf16 so the matmul needs
#     no PE transposes), then an AllGather replicates the bf16 xw' table
#   - phase 2: each core gathers source rows for its edges with bulk
#     indirect DMA (256B bf16 rows), scatter-adds them into per-window PSUM
#     accumulators with one-hot selector matmuls on the PE (selectors built
#     in bf16 on DVE), folds the self-loop in as an identity-selector matmul
#     on the SBUF-resident phase-1 tiles, and runs the whole epilogue
#     (dinv[t] scale + PReLU) as a single ACT op per window.
#
# Host-side work is limited to index preprocessing (edge routing/sorting,
# degree counting, layout shuffles) and the tiny spectral-norm power
# iteration on W.

import numpy as np

import concourse.bacc as bacc
import concourse.bass as bass
import concourse.mybir as mybir
import concourse.tile as tile
from concourse.bass_utils import run_bass_kernel_spmd
from concourse.masks import make_identity

try:
    import ml_dtypes

    BF16 = np.dtype(ml_dtypes.bfloat16)
except ImportError:  # pragma: no cover
    BF16 = None

P = 128
F32 = mybir.dt.float32
BF16_T = mybir.dt.bfloat16
I16 = mybir.dt.int16

# test-harness hooks (ignored in grading): set TRACE=True before calling
# kernel() to capture an NTFF profile; the BassKernelResults lands in
# LAST_RESULT.
TRACE = False
LAST_RESULT = None


def _l2n(v, eps=1e-12):
    return v / (np.linalg.norm(v) + eps)


def _spectral_norm_host(W, u):
    W = W.astype(np.float32)
    u = u.astype(np.float32)
    v = _l2n(W.T @ u)
    u2 = _l2n(W @ v)
    sigma = np.float32(u2 @ (W @ v))
    return W / sigma


def _prep_host(n, edge_index, n_cores, win_group, nbuck, max_call_chunks):
    """Route edges to cores by target and build the SPMD chunk schedule.

    Chunks are 128 edges, each mapping into one 128-target window and one
    source bucket (dma_gather has int16 indices, so the gathered table is
    addressed in buckets of `bucket_rows` rows).  The table in DRAM is laid
    out bucket-major: bucket j holds, for every core c, the rows of c's
    nodes whose local id is in [j*spb, (j+1)*spb) — so bucket j is exactly
    the output of the j-th chunked AllGather and gathers on bucket j can
    start as soon as AG_j lands.  Chunk order: for each super-group of
    `win_group` windows, for each bucket, the chunks of the group's
    windows.  One dma_gather call covers one (group, bucket) run.
    Self-loops are NOT in the edge stream (folded in as identity-selector
    matmuls on the device).
    """
    assert n % n_cores == 0
    npc = n // n_cores
    nwin = -(-npc // P)
    assert npc % nbuck == 0
    spb = npc // nbuck  # rows each core contributes to one bucket
    bucket_rows = spb * n_cores
    assert bucket_rows < 32768

    row = np.ascontiguousarray(edge_index[0]).astype(np.int64)
    col = np.ascontiguousarray(edge_index[1]).astype(np.int64)

    # bucket-major table layout (bucket j = output of chunked AllGather j):
    # node s lives in bucket (s%npc)//spb at row (s//npc)*spb + s%spb
    sbuck = (row % npc) // spb
    srow = (row // npc) * spb + (row % spb)

    # sort all edges by (target window, source bucket) so each (core, window,
    # bucket) run is contiguous; target order within a chunk is free (tloc).
    wkey = (col // npc) * nwin + (col % npc) // P  # global window id
    key = wkey * nbuck + sbuck
    order = np.argsort(key, kind="stable")
    rs = srow[order]
    cs = col[order]
    cwb_sorted = key[order]

    deg = 1.0 + np.bincount(col, minlength=n).astype(np.float64)  # + self loop
    dinv_all = (deg ** -0.5).astype(np.float32)

    # counts per (core, window, bucket)
    cnt = np.bincount(key, minlength=n_cores * nwin * nbuck).reshape(
        n_cores, nwin, nbuck
    )
    kwb = -(-cnt // P)  # chunks per (c, w, b)
    kwb = kwb.max(axis=0)  # [nwin, nbuck] shared schedule

    # chunk order + gather-call runs.  Each call covers chunks of ONE
    # (window, bucket) cell so that every call's padded tail is a run of
    # negative indices; with num_idxs_reg = the per-core true count the DGE
    # skips the tail entirely (no descriptors, no bytes).
    chunk_win = []
    chunk_bucket = []
    call_sizes = []  # chunks per dma_gather call
    call_meta = []  # (w, b, q0) chunk offset of the call within its cell
    for wg in range(0, nwin, win_group):
        ws = range(wg, min(wg + win_group, nwin))
        for b in range(nbuck):
            for w in ws:
                r = int(kwb[w, b])
                if r == 0:
                    continue
                q0 = 0
                while q0 < r:
                    call_sizes.append(min(r - q0, max_call_chunks))
                    call_meta.append((w, b, q0))
                    q0 += max_call_chunks
                chunk_win.extend([w] * r)
                chunk_bucket.extend([b] * r)
    chunk_win = np.asarray(chunk_win)
    chunk_bucket = np.asarray(chunk_bucket)
    nchunks = len(chunk_win)

    # first/last chunk per window in this order
    first_of_win = np.zeros(nchunks, bool)
    last_of_win = np.zeros(nchunks, bool)
    seen = set()
    for j in range(nchunks):
        w = int(chunk_win[j])
        if w not in seen:
            first_of_win[j] = True
            seen.add(w)
    seen = set()
    for j in range(nchunks - 1, -1, -1):
        w = int(chunk_win[j])
        if w not in seen:
            last_of_win[j] = True
            seen.add(w)

    # first destination chunk per (w, b)
    base_by_wb = {}
    for j in range(nchunks):
        key2 = (int(chunk_win[j]), int(chunk_bucket[j]))
        if key2 not in base_by_wb:
            base_by_wb[key2] = j

    # segment boundaries of (core, window, bucket) runs in the sorted list
    seg_lo_idx = np.searchsorted(
        cwb_sorted, np.arange(n_cores * nwin * nbuck), side="left"
    )
    seg_hi_idx = np.searchsorted(
        cwb_sorted, np.arange(n_cores * nwin * nbuck), side="right"
    )

    src_cores = []
    tloc_cores = []
    dinv_cores = []
    gcnt_cores = []
    for c in range(n_cores):
        src_flat = np.full(nchunks * P, -1, np.int16)
        tloc_flat = np.full(nchunks * P, -1.0, np.float32)
        for w in range(nwin):
            for b in range(nbuck):
                if (w, b) not in base_by_wb:
                    continue
                s = c * nwin * nbuck + w * nbuck + b
                i0, i1 = seg_lo_idx[s], seg_hi_idx[s]
                m = i1 - i0
                if m == 0:
                    continue
                d0 = base_by_wb[(w, b)] * P
                src_flat[d0 : d0 + m] = rs[i0:i1].astype(np.int16)
                tloc_flat[d0 : d0 + m] = (cs[i0:i1] - c * npc - w * P).astype(
                    np.float32
                )
        # per-core true (non-negative) index count per call; the padded tail
        # of each call is all -1 and generates no descriptors.  An
        # all-negative call would break the DGE, so force >= 1 real idx.
        gcnt = np.zeros(len(call_sizes), np.int32)
        for ci, ((w, b, q0), r) in enumerate(zip(call_meta, call_sizes)):
            m = int(cnt[c, w, b])
            real = min(max(m - q0 * P, 0), r * P)
            if real == 0:
                s0 = (base_by_wb[(w, b)] + q0) * P
                src_flat[s0] = 0
                real = 1
            gcnt[ci] = real
        gcnt_cores.append(np.ascontiguousarray(gcnt[None, :]))

        # dma_gather idx layout: idx i -> partition i%16, col i//16,
        # replicated over the 8 groups of 16 partitions.
        a = src_flat.reshape(nchunks, 8, 16)  # [j, p//16, p%16]
        a = np.transpose(a, (2, 0, 1)).reshape(16, nchunks * 8)
        src_cores.append(np.ascontiguousarray(np.tile(a, (8, 1))))
        tloc_cores.append(
            np.ascontiguousarray(tloc_flat.reshape(nchunks, P).T.astype(BF16))
        )

        dv = np.zeros(nwin * P, np.float32)
        dv[:npc] = dinv_all[c * npc : (c + 1) * npc]
        dinv_cores.append(np.ascontiguousarray(dv.reshape(nwin, P).T))

    return dict(
        npc=npc,
        nwin=nwin,
        nbuck=nbuck,
        spb=spb,
        bucket_rows=bucket_rows,
        nchunks=nchunks,
        chunk_win=chunk_win,
        chunk_bucket=chunk_bucket,
        call_sizes=call_sizes,
        first_of_win=first_of_win,
        last_of_win=last_of_win,
        src_cores=src_cores,
        tloc_cores=tloc_cores,
        dinv_cores=dinv_cores,
        gcnt_cores=gcnt_cores,
    )


def _build_nc(
    n,
    nfeat,
    nhid,
    n_cores,
    nwin,
    nbuck,
    bucket_rows,
    spb,
    nchunks,
    chunk_win,
    chunk_bucket,
    call_sizes,
    first_of_win,
    last_of_win,
    alpha,
    has_bias,
    gather_bufs=12,
    slab_wins=8,
    dma_scratch=16384,
):
    npc_pad = nwin * P
    npc = n // n_cores
    assert nfeat % P == 0
    nk = nfeat // P  # contraction tiles for x @ W

    nc = bacc.Bacc(
        "TRN2",
        target_bir_lowering=False,
        debug=False,
        enable_asserts=False,
        num_devices=n_cores,
        num_swdge_queues=4,
        dynamic_dma_scratch_size=dma_scratch,
    )

    # x staged pre-transposed+interleaved on host: x_in[p, k, m] = x[m, k*P+p]
    x_in = nc.dram_tensor("x_sh", [P, nk, npc_pad], BF16_T, kind="ExternalInput")
    w_in = nc.dram_tensor("w_sn", [nfeat, nhid], BF16_T, kind="ExternalInput")
    dinv_in = nc.dram_tensor("dinv", [P, nwin], F32, kind="ExternalInput")
    bias_in = nc.dram_tensor("bias_t", [P, nhid], F32, kind="ExternalInput")
    max_call = max(call_sizes)
    iota_in = nc.dram_tensor("iota_t", [P, max_call * P], BF16_T, kind="ExternalInput")
    src_in = nc.dram_tensor("src_idx", [P, nchunks * 8], I16, kind="ExternalInput")
    tloc_in = nc.dram_tensor("tloc", [P, nchunks], BF16_T, kind="ExternalInput")
    gcnt_in = nc.dram_tensor(
        "gcnt", [1, len(call_sizes)], mybir.dt.int32, kind="ExternalInput"
    )
    out_d = nc.dram_tensor("out_sh", [npc_pad, nhid], F32, kind="ExternalOutput")

    assert sum(call_sizes) == nchunks

    with tile.TileContext(nc) as tc:
        with (
            tc.tile_pool(name="consts", bufs=1) as cpool,
            tc.tile_pool(name="dram", bufs=1, space="DRAM") as dpool,
        ):
            # constants
            w_sb = cpool.tile([P, nk, nhid], BF16_T)
            nc.sync.dma_start(w_sb[:], w_in[:].rearrange("(k p) h -> p k h", p=P))
            bias_sb = cpool.tile([P, nhid], F32)
            nc.sync.dma_start(bias_sb[:], bias_in[:])
            iota_sb = cpool.tile([P, max_call * P], BF16_T)
            nc.sync.dma_start(iota_sb[:], iota_in[:])
            dinv_sb = cpool.tile([P, nwin], F32)
            nc.sync.dma_start(dinv_sb[:], dinv_in[:])
            ident = cpool.tile([P, P], BF16_T)
            make_identity(nc, ident[:])
            src_sb = cpool.tile([P, nchunks * 8], I16)
            nc.sync.dma_start(src_sb[:], src_in[:])
            tloc_sb = cpool.tile([P, nchunks], BF16_T)
            nc.sync.dma_start(tloc_sb[:], tloc_in[:])
            gcnt_sb = cpool.tile([1, len(call_sizes)], mybir.dt.int32)
            nc.sync.dma_start(gcnt_sb[:], gcnt_in[:])
            # two banks of 8: the reload of a bank only WAR-depends on
            # gathers 16 calls back, so it never flushes the SWDGE pipeline
            KREG = 8
            NBANK = 2
            gcnt_regs = [
                nc.gpsimd.alloc_register(f"gcnt_reg{i}")
                for i in range(KREG * NBANK)
            ]

            # phase-1 output kept resident in SBUF for the self-loop matmuls
            xwp_sb = cpool.tile([P, nwin * nhid], BF16_T)

            ag_in = dpool.tile([npc, nhid], BF16_T)
            ag_out = [
                dpool.tile(
                    [bucket_rows, nhid],
                    BF16_T,
                    addr_space="Shared",
                    name=f"ag_out{j}",
                )
                for j in range(nbuck)
            ]

            # ---- phase 1: xw' = dinv[s] * (x_s @ W_sn) for owned nodes ----
            # The AllGather is chunked so AG_1..3 overlap the tail of phase 1.
            # AG_0 (the gate for the first gather calls, which are bucket 0)
            # is emitted LAST so no gather overlaps a running collective —
            # concurrent gather+collective DMA trips the HW DMA throttle.
            def _fire_ag(j):
                nc.gpsimd.collective_compute(
                    "AllGather",
                    mybir.AluOpType.bypass,
                    replica_groups=[list(range(n_cores))],
                    ins=[ag_in[j * spb : (j + 1) * spb]],
                    outs=[ag_out[j][:]],
                )

            ag_trigger = {(-(-spb * (j + 1) // P)) - 1: j for j in range(1, nbuck)}
            with (
                tc.tile_pool(name="p1x", bufs=3) as xpool,
                tc.tile_pool(name="p1pm", bufs=4, space="PSUM") as psumXW,
            ):
                for s0 in range(0, nwin, slab_wins):
                    ns = min(slab_wins, nwin - s0)
                    xt = xpool.tile([P, nk, slab_wins * P], BF16_T)
                    nc.sync.dma_start(
                        xt[:, :, : ns * P],
                        x_in[:, :, s0 * P : (s0 + ns) * P],
                    )
                    for wr in range(ns):
                        w = s0 + wr
                        nrow = min(P, npc - w * P)
                        pxw = psumXW.tile([P, nhid], F32)
                        for k in range(nk):
                            nc.tensor.matmul(
                                pxw[:],
                                lhsT=xt[:, k, wr * P : (wr + 1) * P],
                                rhs=w_sb[:, k, :],
                                start=(k == 0),
                                stop=(k == nk - 1),
                            )
                        seg = xwp_sb[:, w * nhid : (w + 1) * nhid]
                        nc.scalar.activation(
                            out=seg,
                            in_=pxw[:],
                            func=mybir.ActivationFunctionType.Copy,
                            scale=dinv_sb[:, w : w + 1],
                        )
                        nc.sync.dma_start(
                            ag_in[w * P : w * P + nrow, :], seg[:nrow]
                        )
                        if w in ag_trigger:
                            _fire_ag(ag_trigger[w])

            _fire_ag(0)

            # ---- phase 2: gather + one-hot matmul scatter-add + epilogue ----
            psum_by_win = {}
            with (
                tc.tile_pool(name="gat", bufs=gather_bufs) as gpool,
                tc.tile_pool(name="sel", bufs=12) as spool,
                tc.tile_pool(name="og", bufs=4) as opool,
                tc.tile_pool(name="acc", bufs=8, space="PSUM") as ppool,
            ):
                j = 0
                for ci, r in enumerate(call_sizes):
                    gbuf = gpool.tile(
                        [P, max_call * nhid], BF16_T, tag="gbuf", name="gbuf"
                    )
                    if ci < gather_bufs:
                        # skipped (negative-idx) tail slots are never written;
                        # zero once so stale SBUF can't inject NaN into matmuls
                        nc.vector.memset(gbuf[:], 0.0)
                    b = int(chunk_bucket[j])
                    bank = ((ci // KREG) % NBANK) * KREG
                    if ci % KREG == 0:
                        nk2 = min(KREG, len(call_sizes) - ci)
                        nc.gpsimd.reg_load(
                            gcnt_regs[bank : bank + nk2],
                            gcnt_sb[0:1, ci : ci + nk2],
                        )
                    nc.gpsimd.dma_gather(
                        gbuf[:, : r * nhid].rearrange("p (k e) -> p k e", e=nhid),
                        ag_out[b][:],
                        src_sb[:, j * 8 : (j + r) * 8],
                        r * P,
                        gcnt_regs[bank + ci % KREG],
                        nhid,
                        queue_num=ci % 4,
                    )
                    # one-hot selectors for the whole call in one DVE op
                    sel_big = spool.tile(
                        [P, max_call * P], BF16_T, tag="sel", name="sel_big"
                    )
                    nc.vector.tensor_tensor(
                        out=sel_big[:, : r * P].rearrange("p (k e) -> p k e", e=P),
                        in0=tloc_sb[:, j : j + r].to_broadcast([P, r, P]),
                        in1=iota_sb[:, : r * P].rearrange("p (k e) -> p k e", e=P),
                        op=mybir.AluOpType.is_equal,
                    )
                    for kk in range(r):
                        w = int(chunk_win[j])
                        if first_of_win[j]:
                            pw = ppool.tile([P, nhid], F32, tag="pw", name="pw")
                            psum_by_win[w] = pw
                            # self-loop: identity selector over the resident
                            # phase-1 tile (start=True resets the bank)
                            nc.tensor.matmul(
                                pw[:],
                                lhsT=ident[:],
                                rhs=xwp_sb[:, w * nhid : (w + 1) * nhid],
                                start=True,
                                stop=False,
                            )
                        pw = psum_by_win[w]
                        nc.tensor.matmul(
                            pw[:],
                            lhsT=sel_big[:, kk * P : (kk + 1) * P],
                            rhs=gbuf[:, kk * nhid : (kk + 1) * nhid],
                            start=False,
                            stop=bool(last_of_win[j]),
                        )
                        if last_of_win[j]:
                            og = opool.tile([P, nhid], F32, tag="og", name="og")
                            nrow = min(P, npc - w * P)
                            if has_bias:
                                nc.scalar.activation(
                                    out=og[:],
                                    in_=pw[:],
                                    func=mybir.ActivationFunctionType.Copy,
                                    scale=dinv_sb[:, w : w + 1],
                                )
                                nc.vector.tensor_tensor(
                                    out=og[:],
                                    in0=og[:],
                                    in1=bias_sb[:],
                                    op=mybir.AluOpType.add,
                                )
                                t2 = opool.tile(
                                    [P, nhid], F32, tag="t2", name="t2"
                                )
                                nc.vector.tensor_scalar(
                                    out=t2[:],
                                    in0=og[:],
                                    scalar1=0.0,
                                    scalar2=float(alpha),
                                    op0=mybir.AluOpType.min,
                                    op1=mybir.AluOpType.mult,
                                )
                                nc.vector.tensor_scalar_max(og[:], og[:], 0.0)
                                nc.vector.tensor_tensor(
                                    out=og[:],
                                    in0=og[:],
                                    in1=t2[:],
                                    op=mybir.AluOpType.add,
                                )
                            else:
                                # out = PReLU(dinv[t] * agg), one ACT op
                                nc.scalar.activation(
                                    out=og[:],
                                    in_=pw[:],
                                    func=mybir.ActivationFunctionType.Prelu,
                                    scale=dinv_sb[:, w : w + 1],
                                    alpha=float(alpha),
                                )
                            nc.sync.dma_start(
                                out_d[w * P : w * P + nrow, :], og[:nrow]
                            )
                        j += 1

    nc.compile()
    return nc


def kernel(**inputs):
    x = np.asarray(inputs["x"], dtype=np.float32)
    edge_index = np.asarray(inputs["edge_index"])
    W = np.asarray(inputs["W"], dtype=np.float32)
    bias = np.asarray(inputs["bias"], dtype=np.float32)
    prelu_a = np.asarray(inputs["prelu_a"], dtype=np.float32)
    u = np.asarray(inputs["u"], dtype=np.float32)

    n, nfeat = x.shape
    nhid = W.shape[1]
    n_cores = 8
    win_group = 4
    nbuck = -(-n // 32767)  # int16 index reach per dma_gather bucket
    alpha = float(prelu_a.reshape(-1)[0])
    has_bias = bool(np.any(bias != 0.0))

    # one dma_gather call must stay under the SWDGE ring carveout
    # (dynamic_dma_scratch_size//16 descriptors); 7 chunks = 896 < 1024
    max_call_chunks = 7
    dma_scratch = 16384

    npc = n // n_cores
    nwin = -(-npc // P)
    npc_pad = nwin * P
    nk = nfeat // P

    w_sn = _spectral_norm_host(W, u)
    prep = _prep_host(n, edge_index, n_cores, win_group, nbuck, max_call_chunks)
    nchunks = prep["nchunks"]

    nc = _build_nc(
        n,
        nfeat,
        nhid,
        n_cores,
        nwin,
        prep["nbuck"],
        prep["bucket_rows"],
        prep["spb"],
        nchunks,
        prep["chunk_win"],
        prep["chunk_bucket"],
        prep["call_sizes"],
        prep["first_of_win"],
        prep["last_of_win"],
        alpha,
        has_bias,
        dma_scratch=dma_scratch,
    )

    bias_t = np.ascontiguousarray(np.tile(bias[None, :], (P, 1)))
    max_call = max(prep["call_sizes"])
    iota_t = np.ascontiguousarray(
        np.tile(
            np.tile(np.arange(P, dtype=np.float32), max_call)[None, :], (P, 1)
        ).astype(BF16)
    )
    w_bf = np.ascontiguousarray(w_sn.astype(BF16))

    in_maps = []
    for c in range(n_cores):
        xp = np.zeros((npc_pad, nfeat), np.float32)
        xp[:npc] = x[c * npc : (c + 1) * npc]
        # x_sh[p, k, m] = xp[m, k*P + p]
        x_sh = np.ascontiguousarray(
            xp.T.reshape(nk, P, npc_pad).transpose(1, 0, 2).astype(BF16)
        )
        in_maps.append(
            {
                "x_sh": x_sh,
                "w_sn": w_bf,
                "dinv": prep["dinv_cores"][c],
                "bias_t": bias_t,
                "iota_t": iota_t,
                "src_idx": prep["src_cores"][c],
                "tloc": prep["tloc_cores"][c],
                "gcnt": prep["gcnt_cores"][c],
            }
        )

    res = run_bass_kernel_spmd(
        nc, in_maps, core_ids=list(range(n_cores)), trace=TRACE
    )
    global LAST_RESULT
    LAST_RESULT = res
    out = np.concatenate(
        [res.results[c]["out_sh"][:npc] for c in range(n_cores)], axis=0
    )
    return out
